# revision 22
# baseline (speedup 1.0000x reference)
"""Trainium2 Bass kernel for Lorentz (hyperboloid) batch norm.

Full-input contract: kernel(**inputs) takes x [64,4096,129] f32, bias [128],
weight scalar; returns y [64,4096,129] f32.  Internally shards batch dim
across 8 NeuronCores (8 batches/core) and runs one Bass/Tile kernel SPMD.

v2 design notes (no-bias fast path; bias!=0 falls back to the v1 builder):
  - x is cast to bf16 on host; all big streams are bf16 (rel-err budget 2e-2,
    measured ~3e-3 end to end).  Stats/coefficients stay f32.
  - per-batch big ops only; all small per-point algebra is batched across the
    8 slabs into [128, 256] tiles (column block b = slab b) to cut
    instruction count (~930 -> ~280) since HW per-instruction overhead
    dominates the measured time.
  - ACT uses only {ln, exp, square, copy, identity} = one table set, zero
    table reloads (sqrt is exp(0.5*ln)).
  - A/B coefficient columns are pre-expanded to dense [128, T*D] bf16 rows on
    ACT so every big DVE multiply runs in 2x_1p packed mode.

Math per slab (reductions over N=4096 points):
  s     = sum_i x_i ;  mu = s / sqrt(max(2*s0^2 - <s,s>, EPS))
  alpha_i = 2*mu0*x_i0 - <mu, x_i>  (clamped >= 1+EPS)
  nu = sqrt(alpha^2-1) ; d = ln(alpha+nu) ; c1 = d/nu
  var = mean(d^2) ; w2 = sqrt(weight/(var+1e-6))
  gamma = -bm0*mu0 ; bet_i = -bm0*x_i0 ; k = c1*(bet - alpha*gamma)/(1-gamma)
  n = max(w2*d, sqrt(EPS)) ; sc = 2*sinh(n)/n ; ch = 2*cosh(n)
  A = sc*(w2/2)*c1 ; B = sc*(w2/2)*(k - c1*alpha) ; C = sc*(w2/2)*k + ch/2
  out_i = A_i*x_i + B_i*mu  (+ C_i*bm0 on component 0)
"""

import numpy as np
from contextlib import ExitStack

import ml_dtypes
import concourse.bacc as bacc
import concourse.tile as tile
from concourse import mybir

AF = mybir.ActivationFunctionType
OP = mybir.AluOpType
F32 = mybir.dt.float32
BF16 = mybir.dt.bfloat16

N_CORES = 8
B_FULL, N, D = 64, 4096, 129
P, T = 128, 32            # N = P*T points per batch; point (p,t) = p*T + t
TD = T * D                # 4128
NB = 8                    # batches per core
EPS = 1e-7
SQRT_EPS = float(np.sqrt(np.float32(EPS)))


def _only_nat_log_exp_tables(arch):
    """Activation tables with every set except natural_log_exp_and_others
    emptied: the ATL chooser assigns ln->natural_log and exp->exp_and_others
    (first containing set), reloading tables on every ln/exp alternation.
    All activation funcs this kernel uses (copy/identity/square/ln/exp) live
    in natural_log_exp_and_others, so forcing that set yields ONE table load.
    Set ids (dict order) are preserved so act_func_set_id stays valid."""
    from concourse.hw_specs import get_activation_tables as _real
    tabs = _real(arch)
    keep = "natural_log_exp_and_others"
    return {name: (s if name == keep else set()) for name, s in tabs.items()}


def build_kernel_v2(bm0: float):
    """No-bias fast path: one core processing NB slabs, x/y in bf16."""
    nc = bacc.Bacc("TRN2", target_bir_lowering=False, debug=False)

    x_d = nc.dram_tensor("x", [NB, N, D], BF16, kind="ExternalInput")
    w_d = nc.dram_tensor("w", [1, 1], F32, kind="ExternalInput")
    onc_d = nc.dram_tensor("ones_col", [P, 1], BF16, kind="ExternalInput")
    oncf_d = nc.dram_tensor("ones_colf", [P, 1], F32, kind="ExternalInput")
    onr_d = nc.dram_tensor("ones_row", [1, P], F32, kind="ExternalInput")
    y_d = nc.dram_tensor("y", [NB, N, D], BF16, kind="ExternalOutput")

    x_r = x_d.ap().rearrange("b (p t) d -> b p (t d)", p=P)
    y_r = y_d.ap().rearrange("b (p t) d -> b p (t d)", p=P)

    with tile.TileContext(nc) as tc, ExitStack() as ctx:
        consts = ctx.enter_context(tc.tile_pool(name="consts", bufs=1))
        # persistent per-slab tiles (live across phases): 8 x tiles + 8 mu reps
        resid = ctx.enter_context(tc.tile_pool(name="resid", bufs=1))
        # batched [P, 256] working set
        bat = ctx.enter_context(tc.tile_pool(name="bat", bufs=1))
        # rotating big scratch
        hsp = ctx.enter_context(tc.tile_pool(name="hsp", bufs=1))
        esp = ctx.enter_context(tc.tile_pool(name="esp", bufs=2))
        osp = ctx.enter_context(tc.tile_pool(name="osp", bufs=2))
        tfp = ctx.enter_context(tc.tile_pool(name="tfp", bufs=1))
        sm = ctx.enter_context(tc.tile_pool(name="sm", bufs=1))
        psS = ctx.enter_context(tc.tile_pool(name="psS", bufs=2, space="PSUM"))
        psM = ctx.enter_context(tc.tile_pool(name="psM", bufs=1, space="PSUM"))
        psR = ctx.enter_context(tc.tile_pool(name="psR", bufs=1, space="PSUM"))

        # ---- constants ----
        wgt = consts.tile([1, 1], F32)
        nc.sync.dma_start(wgt[:], w_d.ap())
        onc = consts.tile([P, 1], BF16)
        nc.sync.dma_start(onc[:], onc_d.ap())
        oncf = consts.tile([P, 1], F32)
        nc.sync.dma_start(oncf[:], oncf_d.ap())
        onr = consts.tile([1, P], F32)
        nc.sync.dma_start(onr[:], onr_d.ap())

        # ---- persistent tiles ----
        xall = resid.tile([P, NB * TD], BF16, tag="xall")
        xb_t = [xall[:, b * TD : (b + 1) * TD] for b in range(NB)]
        mur_all = resid.tile([P, NB * D], BF16, tag="mur_all")
        mur_t = [mur_all[:, b * D : (b + 1) * D] for b in range(NB)]
        s_row = resid.tile([1, NB * D], F32, tag="s_row")   # block b = slab-b sum
        s_r3 = s_row[:].rearrange("q (b d) -> q b d", d=D)
        # batched [P, 256] tiles (column block b = slab b)
        x0_all = bat.tile([P, NB * T], F32, tag="x0")
        pd_all = bat.tile([P, NB * T], F32, tag="pd")
        al_all = bat.tile([P, NB * T], F32, tag="al")

        # =========== phase A: load + slab sums (PE/DMA only) ===========
        def gfold(dstap, dst_w, srcap, src_w):
            # halves-fold each slab block of [P, G*src_w] -> [P, G*dst_w]
            s3 = srcap.rearrange("p (b w) -> p b w", w=src_w)
            nc.vector.tensor_add(
                dstap.rearrange("p (b w) -> p b w", w=dst_w),
                s3[:, :, 0:dst_w], s3[:, :, dst_w:src_w],
            )

        for g in range(0, NB, 2):  # 2-slab chunks; one 3-level fold per chunk
            nc.sync.dma_start(xall[:, g * TD : (g + 1) * TD], x_r[g])
            nc.sync.dma_start(xall[:, (g + 1) * TD : (g + 2) * TD], x_r[g + 1])
            tf1 = tfp.tile([P, TD], BF16, tag="tf1")
            gfold(tf1[:], TD // 2, xall[:, g * TD : (g + 2) * TD], TD)
            tf2 = tfp.tile([P, TD // 2], BF16, tag="tf2")
            gfold(tf2[:], TD // 4, tf1[:], TD // 2)
            tf3 = tfp.tile([P, TD // 4], BF16, tag="tf3")
            gfold(tf3[:], TD // 8, tf2[:], TD // 4)
            for j in range(2):
                b = g + j
                s2 = psS.tile([1, 2 * D], F32, tag="s2")
                for c in range(2):
                    nc.tensor.matmul(
                        s2[:], onc[:],
                        tf3[:, (j * 2 + c) * 2 * D : (j * 2 + c + 1) * 2 * D],
                        start=(c == 0), stop=(c == 1),
                    )
                s2s = sm.tile([1, 2 * D], F32, tag="s2s", bufs=2)
                nc.scalar.copy(s2s[:], s2[:])
                nc.vector.tensor_add(
                    s_row[0:1, b * D : (b + 1) * D], s2s[0:1, 0:D],
                    s2s[0:1, D : 2 * D],
                )

        # =========== batched mu chain on partition-0 rows ===========
        p2 = sm.tile([1, NB * D], F32, tag="p2")
        nc.scalar.square(p2[:], s_row[:])
        ss8 = sm.tile([1, NB], F32, tag="ss8")
        nc.vector.tensor_reduce(
            ss8[:], p2[:].rearrange("q (b d) -> q b d", d=D),
            axis=mybir.AxisListType.X, op=OP.add,
        )
        s0r = sm.tile([1, NB], F32, tag="s0r")
        nc.vector.tensor_copy(s0r[:], s_r3[:, :, 0])
        s0sq = sm.tile([1, NB], F32, tag="s0sq")
        nc.scalar.square(s0sq[:], s0r[:])
        nls = sm.tile([1, NB], F32, tag="nls")
        nc.vector.scalar_tensor_tensor(
            out=nls[:], in0=s0sq[:], scalar=2.0, in1=ss8[:],
            op0=OP.mult, op1=OP.subtract,
        )
        nc.vector.tensor_scalar_max(nls[:], nls[:], EPS)
        lg8 = sm.tile([1, NB], F32, tag="lg8")
        nc.scalar.activation(lg8[:], nls[:], AF.Ln)
        rinv = sm.tile([1, NB], F32, tag="rinv")
        nc.scalar.activation(rinv[:], lg8[:], AF.Exp, scale=-0.5)
        mu_row = sm.tile([1, NB * D], F32, tag="mu_row")
        nc.vector.tensor_tensor(
            mu_row[:].rearrange("q (b d) -> q b d", d=D), s_r3,
            rinv[:].unsqueeze(2).broadcast_to([1, NB, D]), OP.mult,
        )
        mu0 = sm.tile([1, NB], F32, tag="mu0")
        nc.vector.tensor_mul(mu0[:], s0r[:], rinv[:])
        m2row = sm.tile([1, NB], F32, tag="m2row")
        nc.vector.tensor_scalar_mul(m2row[:], mu0[:], 2.0)
        srow4 = sm.tile([1, 4 * NB], F32, tag="srow4")
        nc.vector.tensor_scalar_mul(srow4[:, 0:NB], mu0[:], float(bm0))  # -gamma
        denrow = sm.tile([1, NB], F32, tag="denrow")
        nc.vector.tensor_scalar(
            denrow[:], mu0[:], float(bm0), 1.0, op0=OP.mult, op1=OP.add
        )
        nc.vector.reciprocal(srow4[:, NB : 2 * NB], denrow[:])  # 1/(1-gamma)

        # [P, 256] block tensor of 2*mu0 for the batched alpha op
        m2brow = sm.tile([1, NB * T], F32, tag="m2brow")
        nc.vector.tensor_copy(
            m2brow[:].rearrange("q (b t) -> q b t", t=T),
            m2row[:].unsqueeze(2).broadcast_to([1, NB, T]),
        )
        m2_ps = psR.tile([P, NB * T], F32, tag="m2ps")
        nc.tensor.matmul(m2_ps[:], onr[:], m2brow[:], start=True, stop=True)
        m2blk = consts.tile([P, NB * T], F32)
        nc.scalar.copy(m2blk[:], m2_ps[:])

        # mu replicated across partitions, cast bf16: three <=512-wide matmuls
        for third in range(3):
            cw = NB * D // 3  # 344
            mp = psM.tile([P, cw], F32, tag="mp")
            nc.tensor.matmul(
                mp[:], onr[:], mu_row[0:1, third * cw : (third + 1) * cw],
                start=True, stop=True,
            )
            nc.scalar.copy(mur_all[:, third * cw : (third + 1) * cw], mp[:])

        # =========== phase B: per-slab alpha ===========
        for b in range(NB):
            bs = slice(b * T, (b + 1) * T)
            xb3 = xb_t[b].rearrange("p (t d) -> p t d", d=D)
            h = hsp.tile([P, TD], BF16, tag="h")
            h3 = h[:].rearrange("p (t d) -> p t d", d=D)
            mu_b = mur_t[b].unsqueeze(1).broadcast_to([P, T, D])
            nc.vector.tensor_tensor(h3, xb3, mu_b, OP.mult)  # 2x packed
            nc.vector.tensor_reduce(
                pd_all[:, bs], h3, axis=mybir.AxisListType.X, op=OP.add
            )

        # =========== batched coefficient pipeline on [P, 256] ===========
        W = NB * T

        def bt(tag):
            return bat.tile([P, W], F32, tag=tag, name=tag)

        # x0 for all slabs in one strided copy: [P, (b t d)] -> [P, (b t)]
        nc.vector.tensor_copy(
            x0_all[:].rearrange("p (b t) -> p b t", t=T),
            xall[:].rearrange("p (b t d) -> p b t d", t=T, d=D)[:, :, :, 0],
        )
        nc.vector.tensor_mul(al_all[:], x0_all[:], m2blk[:])
        nc.vector.tensor_sub(al_all[:], al_all[:], pd_all[:])
        nc.vector.tensor_scalar_max(al_all[:], al_all[:], 1.0 + EPS)
        sq = bt("sq")
        nc.scalar.square(sq[:], al_all[:])
        am1 = bt("am1")
        nc.vector.tensor_scalar(am1[:], sq[:], -1.0, EPS, op0=OP.add, op1=OP.max)
        lnA = bt("lnA")
        nc.scalar.activation(lnA[:], am1[:], AF.Ln)
        nu = bt("nu")
        nc.scalar.activation(nu[:], lnA[:], AF.Exp, scale=0.5)
        dsum = bt("dsum")
        nc.vector.tensor_add(dsum[:], al_all[:], nu[:])
        dd = bt("dd")
        nc.scalar.activation(dd[:], dsum[:], AF.Ln)
        rnu = bt("rnu")
        nc.vector.reciprocal(rnu[:], nu[:])
        c1 = bt("c1")
        nc.vector.tensor_mul(c1[:], dd[:], rnu[:])

        # var per slab: d^2 accumulated over each 32-col block, then col-sum
        ds1 = sm.tile([P, NB], F32, tag="ds1")
        scrW = bt("scrW")
        for b in range(NB):
            bs = slice(b * T, (b + 1) * T)
            nc.scalar.activation(
                scrW[:, bs], dd[:, bs], AF.Square, accum_out=ds1[:, b : b + 1]
            )
        var_ps = psR.tile([1, NB], F32, tag="var")
        nc.tensor.matmul(var_ps[:], oncf[:], ds1[:], start=True, stop=True)
        varm = sm.tile([1, NB], F32, tag="varm")
        nc.scalar.activation(varm[:], var_ps[:], AF.Copy, bias=1e-6, scale=1.0 / float(N))
        rv = sm.tile([1, NB], F32, tag="rv")
        nc.vector.reciprocal(rv[:], varm[:])
        w2sq = sm.tile([1, NB], F32, tag="w2sq")
        nc.vector.tensor_scalar_mul(w2sq[:], rv[:], wgt[:])
        lw = sm.tile([1, NB], F32, tag="lw")
        nc.scalar.activation(lw[:], w2sq[:], AF.Ln)
        nc.scalar.activation(srow4[:, 2 * NB : 3 * NB], lw[:], AF.Exp, scale=0.5)
        nc.vector.tensor_scalar_mul(
            srow4[:, 3 * NB : 4 * NB], srow4[:, 2 * NB : 3 * NB], 0.5
        )

        # block-constant [P, 256] tensors for {gneg, invd, w2, w2h}: the four
        # producers wrote slices of srow4 [1, 32]; one broadcast row + two
        # <=512-wide replication matmuls produce all four [P, 256] blocks.
        brow = sm.tile([1, 4 * W], F32, tag="brow")
        nc.vector.tensor_copy(
            brow[:].rearrange("q (s t) -> q s t", t=T),
            srow4[:].unsqueeze(2).broadcast_to([1, 4 * NB, T]),
        )
        blk_all = bat.tile([P, 4 * W], F32, tag="blk_all")
        for half in range(2):
            ps = psR.tile([P, 2 * W], F32, tag="blkps", name=f"blkps{half}")
            nc.tensor.matmul(
                ps[:], onr[:], brow[0:1, half * 2 * W : (half + 1) * 2 * W],
                start=True, stop=True,
            )
            nc.scalar.copy(blk_all[:, half * 2 * W : (half + 1) * 2 * W], ps[:])
        gnegb = blk_all[:, 0:W]
        invdb = blk_all[:, W : 2 * W]
        w2b = blk_all[:, 2 * W : 3 * W]
        w2hb = blk_all[:, 3 * W : 4 * W]

        nn = bt("nn")
        nc.vector.tensor_mul(nn[:], dd[:], w2b[:])
        nc.vector.tensor_scalar_max(nn[:], nn[:], SQRT_EPS)
        ee = bt("ee")
        nc.scalar.activation(ee[:], nn[:], AF.Exp)
        em = bt("em")
        nc.scalar.activation(em[:], nn[:], AF.Exp, scale=-1.0)
        rn = bt("rn")
        nc.vector.reciprocal(rn[:], nn[:])
        sh = bt("sh")
        nc.vector.tensor_sub(sh[:], ee[:], em[:])
        ch = bt("ch")
        nc.vector.tensor_add(ch[:], ee[:], em[:])
        sc = bt("sc")
        nc.vector.tensor_mul(sc[:], sh[:], rn[:])       # 2*sinh(n)/n
        scW = bt("scW")
        nc.vector.tensor_mul(scW[:], sc[:], w2hb[:])    # sc*w2/2

        bet = bt("bet")
        nc.vector.tensor_scalar_mul(bet[:], x0_all[:], float(-bm0))
        t1 = bt("t1")
        nc.vector.tensor_mul(t1[:], al_all[:], gnegb[:])
        nc.vector.tensor_add(t1[:], t1[:], bet[:])
        k1 = bt("k1")
        nc.vector.tensor_mul(k1[:], t1[:], invdb[:])
        kf = bt("kf")
        nc.vector.tensor_mul(kf[:], k1[:], c1[:])

        Af = bt("Af")
        nc.vector.tensor_mul(Af[:], scW[:], c1[:])
        Cpre = bt("Cpre")
        nc.vector.tensor_mul(Cpre[:], scW[:], kf[:])
        # B = scW*(kf - c1*alpha) = Cpre - Af*alpha
        Bf = bt("Bf")
        nc.vector.tensor_mul(Bf[:], Af[:], al_all[:])
        nc.vector.tensor_sub(Bf[:], Cpre[:], Bf[:])
        Cf = bt("Cf")
        nc.vector.scalar_tensor_tensor(
            out=Cf[:], in0=ch[:], scalar=0.5, in1=Cpre[:], op0=OP.mult, op1=OP.add
        )
        Ab = bat.tile([P, W], BF16, tag="Ab")
        nc.vector.tensor_copy(Ab[:], Af[:])
        Bb = bat.tile([P, W], BF16, tag="Bb")
        nc.vector.tensor_copy(Bb[:], Bf[:])
        Cb = bat.tile([P, W], BF16, tag="Cb")
        nc.vector.tensor_copy(Cb[:], Cf[:])

        # =========== phase C: per-slab combine + store ===========
        for b in range(NB):
            bs = slice(b * T, (b + 1) * T)
            xb3 = xb_t[b].rearrange("p (t d) -> p t d", d=D)
            # dense bf16 expansions of the A/B columns (ACT, off DVE)
            aexp = esp.tile([P, TD], BF16, tag="aexp")
            a3 = aexp[:].rearrange("p (t d) -> p t d", d=D)
            nc.scalar.copy(a3, Ab[:, bs].unsqueeze(2).broadcast_to([P, T, D]))
            bexp = esp.tile([P, TD], BF16, tag="bexp")
            b3 = bexp[:].rearrange("p (t d) -> p t d", d=D)
            nc.gpsimd.tensor_copy(b3, Bb[:, bs].unsqueeze(2).broadcast_to([P, T, D]))

            r = hsp.tile([P, TD], BF16, tag="r")
            r3 = r[:].rearrange("p (t d) -> p t d", d=D)
            mu_b = mur_t[b].unsqueeze(1).broadcast_to([P, T, D])
            nc.vector.tensor_tensor(r3, b3, mu_b, OP.mult)       # 2x
            out_sb = osp.tile([P, TD], BF16, tag="o")
            o3 = out_sb[:].rearrange("p (t d) -> p t d", d=D)
            nc.vector.tensor_tensor(o3, xb3, a3, OP.mult)        # 2x
            nc.vector.tensor_add(out_sb[:], out_sb[:], r[:])     # 2x
            nc.vector.scalar_tensor_tensor(
                out=o3[:, :, 0], in0=Cb[:, bs], scalar=float(bm0), in1=o3[:, :, 0],
                op0=OP.mult, op1=OP.add,
            )
            nc.sync.dma_start(y_r[b], out_sb[:])

    real_tables = bacc.get_activation_tables
    bacc.get_activation_tables = _only_nat_log_exp_tables
    try:
        nc.compile()
    finally:
        bacc.get_activation_tables = real_tables
    return nc


# ---------------------------------------------------------------------------
# v1 builder (general-bias fallback) -- identical to the original baseline.
# ---------------------------------------------------------------------------

def build_kernel_v1(n_batch: int, has_bias: bool, bm0: float):
    nc = bacc.Bacc("TRN2", target_bir_lowering=False, debug=False)

    x_d = nc.dram_tensor("x", [n_batch, N, D], F32, kind="ExternalInput")
    bm_d = nc.dram_tensor("bm", [1, D], F32, kind="ExternalInput")
    bmt_d = nc.dram_tensor("bmt", [1, D], F32, kind="ExternalInput")
    w_d = nc.dram_tensor("w", [1, 1], F32, kind="ExternalInput")
    onc_d = nc.dram_tensor("ones_col", [P, 1], F32, kind="ExternalInput")
    onr_d = nc.dram_tensor("ones_row", [1, P], F32, kind="ExternalInput")
    idn_d = nc.dram_tensor("ident", [P, P], F32, kind="ExternalInput")
    y_d = nc.dram_tensor("y", [n_batch, N, D], F32, kind="ExternalOutput")

    x_r = x_d.ap().rearrange("b (p t) d -> b p (t d)", p=P)
    y_r = y_d.ap().rearrange("b (p t) d -> b p (t d)", p=P)

    with tile.TileContext(nc) as tc, ExitStack() as ctx:
        consts = ctx.enter_context(tc.tile_pool(name="consts", bufs=1))
        xpool = ctx.enter_context(tc.tile_pool(name="xp", bufs=2))
        opool = ctx.enter_context(tc.tile_pool(name="op", bufs=3))
        mpool = ctx.enter_context(tc.tile_pool(name="mp", bufs=2))
        pp = ctx.enter_context(tc.tile_pool(name="pp", bufs=3))
        sm = ctx.enter_context(tc.tile_pool(name="sm", bufs=3))
        btp = ctx.enter_context(tc.tile_pool(name="btp", bufs=2))
        psA = ctx.enter_context(tc.tile_pool(name="psA", bufs=3, space="PSUM"))

        bm = consts.tile([1, D], F32)
        nc.sync.dma_start(bm[:], bm_d.ap())
        bmt = consts.tile([1, D], F32)
        nc.sync.dma_start(bmt[:], bmt_d.ap())
        wgt = consts.tile([1, 1], F32)
        nc.sync.dma_start(wgt[:], w_d.ap())
        onc = consts.tile([P, 1], F32)
        nc.sync.dma_start(onc[:], onc_d.ap())
        onr = consts.tile([1, P], F32)
        nc.sync.dma_start(onr[:], onr_d.ap())
        idn = consts.tile([P, P], F32)
        nc.sync.dma_start(idn[:], idn_d.ap())

        if has_bias:
            bmt_ps = psA.tile([P, D], F32, tag="ps_small")
            nc.tensor.matmul(bmt_ps[:], onr[:], bmt[:], start=True, stop=True)
            bmt_rep = consts.tile([P, D], F32)
            nc.scalar.copy(bmt_rep[:], bmt_ps[:])
            bm_ps = psA.tile([P, D], F32, tag="ps_small")
            nc.tensor.matmul(bm_ps[:], onr[:], bm[:], start=True, stop=True)
            bm_rep = consts.tile([P, D], F32)
            nc.scalar.copy(bm_rep[:], bm_ps[:])

        def stage1(b):
            st = {}
            xb = xpool.tile([P, T * D], F32)
            nc.sync.dma_start(xb[:], x_r[b])
            xb3 = xb[:].rearrange("p (t d) -> p t d", d=D)
            st["xb3"] = xb3
            out_sb = opool.tile([P, T * D], F32)
            st["out_sb"] = out_sb
            h13 = out_sb[:].rearrange("p (t d) -> p t d", d=D)

            s_ps = psA.tile([1, D], F32, tag="ps_small")
            for t in range(T):
                nc.tensor.matmul(
                    s_ps[:], onc[:], xb3[:, t, :], start=(t == 0), stop=(t == T - 1)
                )
            s_sb = sm.tile([1, D], F32)
            nc.scalar.copy(s_sb[:], s_ps[:])

            scr_d = sm.tile([1, D], F32)
            ssum = sm.tile([1, 1], F32)
            nc.vector.tensor_mul(scr_d[:], s_sb[:], s_sb[:])
            nc.vector.tensor_reduce(
                ssum[:], scr_d[:], axis=mybir.AxisListType.X, op=OP.add
            )
            s0sq = sm.tile([1, 1], F32)
            nc.scalar.square(s0sq[:], s_sb[0:1, 0:1])
            nls = sm.tile([1, 1], F32)
            nc.vector.scalar_tensor_tensor(
                out=nls[:], in0=s0sq[:], scalar=2.0, in1=ssum[:],
                op0=OP.mult, op1=OP.subtract,
            )
            nc.vector.tensor_scalar_max(nls[:], nls[:], EPS)
            rls = sm.tile([1, 1], F32)
            nc.vector.reciprocal(rls[:], nls[:])
            rsq = sm.tile([1, 1], F32)
            nc.scalar.sqrt(rsq[:], rls[:])
            mu = sm.tile([1, D], F32)
            nc.vector.tensor_scalar_mul(mu[:], s_sb[:], rsq[:])

            mu_ps = psA.tile([P, D], F32, tag="ps_small")
            nc.tensor.matmul(mu_ps[:], onr[:], mu[:], start=True, stop=True)
            mu_rep = mpool.tile([P, D], F32)
            nc.scalar.copy(mu_rep[:], mu_ps[:])

            stageA = sm.tile([1, 3], F32)
            nc.scalar.mul(stageA[:, 0:1], mu[0:1, 0:1], 2.0)
            scr_d2 = sm.tile([1, D], F32)
            nc.vector.tensor_mul(scr_d2[:], mu[:], bmt[:])
            g_pos = sm.tile([1, 1], F32)
            nc.vector.tensor_reduce(
                g_pos[:], scr_d2[:], axis=mybir.AxisListType.X, op=OP.add
            )
            nc.scalar.mul(stageA[:, 1:2], g_pos[:], -1.0)
            one_mg = sm.tile([1, 1], F32)
            nc.scalar.activation(one_mg[:], g_pos[:], AF.Identity, scale=-1.0, bias=1.0)
            nc.vector.reciprocal(stageA[:, 2:3], one_mg[:])
            repsA_ps = psA.tile([P, 3], F32, tag="ps_small")
            nc.tensor.matmul(repsA_ps[:], onr[:], stageA[:], start=True, stop=True)
            repsA = pp.tile([P, 3], F32)
            nc.scalar.copy(repsA[:], repsA_ps[:])
            mu0x2_rep = repsA[:, 0:1]
            ngam_rep = repsA[:, 1:2]
            invden_rep = repsA[:, 2:3]

            mu_b = mu_rep[:].unsqueeze(1).broadcast_to([P, T, D])
            nc.vector.tensor_tensor(h13, xb3, mu_b, OP.mult)

            pdot = pp.tile([P, T], F32)
            scrA = sm.tile([P, D], F32)
            nc.vector.tensor_reduce(
                pdot[:], h13, axis=mybir.AxisListType.X, op=OP.add
            )

            x0t = pp.tile([P, T], F32)
            nc.scalar.copy(x0t[:], xb3[:, :, 0])
            alpha = pp.tile([P, T], F32)
            nc.vector.scalar_tensor_tensor(
                out=alpha[:], in0=x0t[:], scalar=mu0x2_rep, in1=pdot[:],
                op0=OP.mult, op1=OP.subtract,
            )
            nc.vector.tensor_scalar_max(alpha[:], alpha[:], 1.0 + EPS)

            sq = pp.tile([P, T], F32)
            nc.scalar.square(sq[:], alpha[:])
            am1 = pp.tile([P, T], F32)
            nc.vector.tensor_scalar_add(am1[:], sq[:], -1.0)
            nc.vector.tensor_scalar_max(am1[:], am1[:], EPS)
            nu = pp.tile([P, T], F32)
            nc.scalar.sqrt(nu[:], am1[:])
            dsum = pp.tile([P, T], F32)
            nc.vector.tensor_add(dsum[:], alpha[:], nu[:])
            dd = pp.tile([P, T], F32)
            nc.scalar.activation(dd[:], dsum[:], AF.Ln)
            rnu = pp.tile([P, T], F32)
            nc.vector.reciprocal(rnu[:], nu[:])
            c1 = pp.tile([P, T], F32)
            nc.vector.tensor_mul(c1[:], dd[:], rnu[:])

            scrT = pp.tile([P, T], F32)
            ds1 = pp.tile([P, 1], F32)
            nc.scalar.activation(scrT[:], dd[:], AF.Square, accum_out=ds1[:])
            var_ps = psA.tile([1, 1], F32, tag="ps_small")
            nc.tensor.matmul(var_ps[:], onc[:], ds1[:], start=True, stop=True)
            varm = sm.tile([1, 1], F32)
            nc.scalar.activation(
                varm[:], var_ps[:], AF.Copy, bias=1e-6, scale=1.0 / float(N)
            )
            rv = sm.tile([1, 1], F32)
            nc.vector.reciprocal(rv[:], varm[:])
            w2sq = sm.tile([1, 1], F32)
            nc.vector.tensor_mul(w2sq[:], rv[:], wgt[:])
            stageB = sm.tile([1, 2], F32)
            nc.scalar.sqrt(stageB[:, 0:1], w2sq[:])
            nc.scalar.mul(stageB[:, 1:2], stageB[:, 0:1], 0.5)
            repsB_ps = psA.tile([P, 2], F32, tag="ps_small")
            nc.tensor.matmul(repsB_ps[:], onr[:], stageB[:], start=True, stop=True)
            repsB = pp.tile([P, 2], F32)
            nc.scalar.copy(repsB[:], repsB_ps[:])
            w2_rep = repsB[:, 0:1]
            w2h_rep = repsB[:, 1:2]

            bet = pp.tile([P, T], F32)
            if has_bias:
                hb = btp.tile([P, T * D], F32, tag="hb")
                hb3 = hb[:].rearrange("p (t d) -> p t d", d=D)
                bmt_b = bmt_rep[:].unsqueeze(1).broadcast_to([P, T, D])
                nc.vector.tensor_tensor(hb3, xb3, bmt_b, OP.mult)
                for t in range(T):
                    nc.scalar.activation(
                        scrA[:], hb3[:, t, :], AF.Copy, accum_out=bet[:, t : t + 1]
                    )
            else:
                nc.vector.tensor_scalar_mul(bet[:], x0t[:], float(-bm0))

            t1 = pp.tile([P, T], F32)
            nc.vector.scalar_tensor_tensor(
                out=t1[:], in0=alpha[:], scalar=ngam_rep, in1=bet[:],
                op0=OP.mult, op1=OP.add,
            )
            k1 = pp.tile([P, T], F32)
            nc.vector.tensor_scalar_mul(k1[:], t1[:], invden_rep)
            kf = pp.tile([P, T], F32)
            nc.vector.tensor_mul(kf[:], k1[:], c1[:])

            nn = pp.tile([P, T], F32)
            nc.vector.tensor_scalar_mul(nn[:], dd[:], w2_rep)
            nc.vector.tensor_scalar_max(nn[:], nn[:], SQRT_EPS)
            ee = pp.tile([P, T], F32)
            nc.scalar.activation(ee[:], nn[:], AF.Exp)
            em = pp.tile([P, T], F32)
            nc.scalar.activation(em[:], nn[:], AF.Exp, scale=-1.0)
            rn = pp.tile([P, T], F32)
            nc.vector.reciprocal(rn[:], nn[:])
            sh = pp.tile([P, T], F32)
            nc.vector.tensor_sub(sh[:], ee[:], em[:])
            sc = pp.tile([P, T], F32)
            nc.vector.tensor_mul(sc[:], sh[:], rn[:])
            ch = pp.tile([P, T], F32)
            nc.vector.tensor_add(ch[:], ee[:], em[:])

            Aco = pp.tile([P, T], F32)
            a3 = pp.tile([P, T], F32)
            nc.vector.tensor_scalar_mul(a3[:], c1[:], w2h_rep)
            nc.vector.tensor_mul(Aco[:], sc[:], a3[:])
            st["Aco"] = Aco

            ca = pp.tile([P, T], F32)
            nc.vector.tensor_mul(ca[:], c1[:], alpha[:])
            kc = pp.tile([P, T], F32)
            nc.vector.tensor_sub(kc[:], kf[:], ca[:])
            b3 = pp.tile([P, T], F32)
            nc.vector.tensor_scalar_mul(b3[:], kc[:], w2h_rep)
            Bco = pp.tile([P, T], F32)
            nc.vector.tensor_mul(Bco[:], sc[:], b3[:])
            c3 = pp.tile([P, T], F32)
            nc.vector.tensor_scalar_mul(c3[:], kf[:], w2h_rep)
            c0 = pp.tile([P, T], F32)
            nc.vector.tensor_mul(c0[:], sc[:], c3[:])
            Cco = pp.tile([P, T], F32)
            nc.vector.scalar_tensor_tensor(
                out=Cco[:], in0=ch[:], scalar=0.5, in1=c0[:],
                op0=OP.mult, op1=OP.add,
            )
            st["Bco"] = Bco
            st["Cco"] = Cco
            st["mu_rep"] = mu_rep
            st["b"] = b
            return st

        def stage2(st):
            xb3 = st["xb3"]
            o3 = st["out_sb"][:].rearrange("p (t d) -> p t d", d=D)
            mu_rep, Aco, Bco, Cco, b = st["mu_rep"], st["Aco"], st["Bco"], st["Cco"], st["b"]
            rr = xpool.tile([P, T * D], F32, tag="rr")
            r3 = rr[:].rearrange("p (t d) -> p t d", d=D)
            A_b = Aco[:].unsqueeze(2).broadcast_to([P, T, D])
            B_b = Bco[:].unsqueeze(2).broadcast_to([P, T, D])
            mu_b2 = mu_rep[:].unsqueeze(1).broadcast_to([P, T, D])
            nc.vector.tensor_tensor(r3, B_b, mu_b2, OP.mult)
            nc.vector.tensor_tensor(o3, xb3, A_b, OP.mult)
            nc.vector.tensor_tensor(o3, o3, r3, OP.add)
            if has_bias:
                C_b = Cco[:].unsqueeze(2).broadcast_to([P, T, D])
                bm_b = bm_rep[:].unsqueeze(1).broadcast_to([P, T, D])
                nc.vector.tensor_tensor(r3, C_b, bm_b, OP.mult)
                nc.vector.tensor_tensor(o3, o3, r3, OP.add)
            else:
                nc.vector.scalar_tensor_tensor(
                    out=o3[:, :, 0], in0=Cco[:], scalar=float(bm0), in1=o3[:, :, 0],
                    op0=OP.mult, op1=OP.add,
                )
            nc.sync.dma_start(y_r[b], st["out_sb"][:])

        prev = None
        for b in range(n_batch):
            cur = stage1(b)
            if prev is not None:
                stage2(prev)
            prev = cur
        stage2(prev)

    nc.compile()
    return nc


def _host_bias_manifold(bias: np.ndarray):
    b32 = np.asarray(bias, dtype=np.float32)
    sq = np.float32(np.sum(b32 * b32, dtype=np.float32))
    nrm2 = np.maximum(sq, np.float32(EPS))
    n = np.sqrt(nrm2)
    bm = np.zeros(D, dtype=np.float32)
    bm[0] = np.cosh(n)
    bm[1:] = (np.sinh(n) / n) * b32
    return bm


_CACHE = {}


def _get_nc_v2(bm0):
    if "v2" not in _CACHE:
        _CACHE["v2"] = build_kernel_v2(bm0)
    return _CACHE["v2"]


def _get_nc_v1(n_batch, has_bias, bm0):
    key = ("v1", n_batch, has_bias)
    if key not in _CACHE:
        _CACHE[key] = build_kernel_v1(n_batch, has_bias, bm0)
    return _CACHE[key]


def _make_in_maps_v2(x, weight):
    xb = x.astype(ml_dtypes.bfloat16)
    b_sh = x.shape[0] // N_CORES
    common = {
        "w": np.asarray(weight, dtype=np.float32).reshape(1, 1),
        "ones_col": np.ones((P, 1), dtype=ml_dtypes.bfloat16),
        "ones_colf": np.ones((P, 1), dtype=np.float32),
        "ones_row": np.ones((1, P), dtype=np.float32),
    }
    return [
        {"x": np.ascontiguousarray(xb[c * b_sh : (c + 1) * b_sh]), **common}
        for c in range(N_CORES)
    ]


def _make_in_maps_v1(x, bias, weight):
    bias = np.asarray(bias, dtype=np.float32)
    bm = _host_bias_manifold(bias)
    bmt = bm.copy()
    bmt[0] = -bmt[0]
    b_sh = x.shape[0] // N_CORES
    common = {
        "bm": bm.reshape(1, D),
        "bmt": bmt.reshape(1, D),
        "w": np.asarray(weight, dtype=np.float32).reshape(1, 1),
        "ones_col": np.ones((P, 1), dtype=np.float32),
        "ones_row": np.ones((1, P), dtype=np.float32),
        "ident": np.eye(P, dtype=np.float32),
    }
    return [
        {"x": np.ascontiguousarray(x[c * b_sh : (c + 1) * b_sh]), **common}
        for c in range(N_CORES)
    ]


def kernel(x, bias, weight):
    from concourse.bass_utils import run_bass_kernel_spmd

    x = np.ascontiguousarray(np.asarray(x, dtype=np.float32))
    assert x.shape == (B_FULL, N, D), x.shape
    bias = np.asarray(bias, dtype=np.float32)
    has_bias = bool(np.any(bias != 0))
    bm = _host_bias_manifold(bias)
    if has_bias:
        in_maps = _make_in_maps_v1(x, bias, weight)
        nc = _get_nc_v1(B_FULL // N_CORES, True, float(bm[0]))
    else:
        in_maps = _make_in_maps_v2(x, weight)
        nc = _get_nc_v2(float(bm[0]))
    res = run_bass_kernel_spmd(nc, in_maps, core_ids=list(range(N_CORES)))
    y = np.concatenate(
        [res.results[c]["y"].astype(np.float32) for c in range(N_CORES)], axis=0
    )
    return y


# revision 25
# speedup vs baseline: 1.0234x; 1.0234x over previous
"""Trainium2 Bass kernel for Lorentz (hyperboloid) batch norm.

Full-input contract: kernel(**inputs) takes x [64,4096,129] f32, bias [128],
weight scalar; returns y [64,4096,129] f32.  Internally shards batch dim
across 8 NeuronCores (8 batches/core) and runs one Bass/Tile kernel SPMD.

v2 design notes (no-bias fast path; bias!=0 falls back to the v1 builder):
  - x is cast to bf16 on host; all big streams are bf16 (rel-err budget 2e-2,
    measured ~3e-3 end to end).  Stats/coefficients stay f32.
  - per-batch big ops only; all small per-point algebra is batched across the
    8 slabs into [128, 256] tiles (column block b = slab b) to cut
    instruction count (~930 -> ~280) since HW per-instruction overhead
    dominates the measured time.
  - ACT uses only {ln, exp, square, copy, identity} = one table set, zero
    table reloads (sqrt is exp(0.5*ln)).
  - A/B coefficient columns are pre-expanded to dense [128, T*D] bf16 rows on
    ACT so every big DVE multiply runs in 2x_1p packed mode.

Math per slab (reductions over N=4096 points):
  s     = sum_i x_i ;  mu = s / sqrt(max(2*s0^2 - <s,s>, EPS))
  alpha_i = 2*mu0*x_i0 - <mu, x_i>  (clamped >= 1+EPS)
  nu = sqrt(alpha^2-1) ; d = ln(alpha+nu) ; c1 = d/nu
  var = mean(d^2) ; w2 = sqrt(weight/(var+1e-6))
  gamma = -bm0*mu0 ; bet_i = -bm0*x_i0 ; k = c1*(bet - alpha*gamma)/(1-gamma)
  n = max(w2*d, sqrt(EPS)) ; sc = 2*sinh(n)/n ; ch = 2*cosh(n)
  A = sc*(w2/2)*c1 ; B = sc*(w2/2)*(k - c1*alpha) ; C = sc*(w2/2)*k + ch/2
  out_i = A_i*x_i + B_i*mu  (+ C_i*bm0 on component 0)
"""

import numpy as np
from contextlib import ExitStack

import ml_dtypes
import concourse.bacc as bacc
import concourse.tile as tile
from concourse import mybir

AF = mybir.ActivationFunctionType
OP = mybir.AluOpType
F32 = mybir.dt.float32
BF16 = mybir.dt.bfloat16

N_CORES = 8
B_FULL, N, D = 64, 4096, 129
P, T = 128, 32            # N = P*T points per batch; point (p,t) = p*T + t
TD = T * D                # 4128
NB = 8                    # batches per core
EPS = 1e-7
SQRT_EPS = float(np.sqrt(np.float32(EPS)))


def _only_nat_log_exp_tables(arch):
    """Activation tables with every set except natural_log_exp_and_others
    emptied: the ATL chooser assigns ln->natural_log and exp->exp_and_others
    (first containing set), reloading tables on every ln/exp alternation.
    All activation funcs this kernel uses (copy/identity/square/ln/exp) live
    in natural_log_exp_and_others, so forcing that set yields ONE table load.
    Set ids (dict order) are preserved so act_func_set_id stays valid."""
    from concourse.hw_specs import get_activation_tables as _real
    tabs = _real(arch)
    keep = "natural_log_exp_and_others"
    return {name: (s if name == keep else set()) for name, s in tabs.items()}


def build_kernel_v2(bm0: float):
    """No-bias fast path: one core processing NB slabs, x/y in bf16."""
    nc = bacc.Bacc("TRN2", target_bir_lowering=False, debug=False)

    x_d = nc.dram_tensor("x", [NB, N, D], BF16, kind="ExternalInput")
    w_d = nc.dram_tensor("w", [1, 1], F32, kind="ExternalInput")
    onc_d = nc.dram_tensor("ones_col", [P, 1], BF16, kind="ExternalInput")
    oncf_d = nc.dram_tensor("ones_colf", [P, 1], F32, kind="ExternalInput")
    onr_d = nc.dram_tensor("ones_row", [1, P], F32, kind="ExternalInput")
    onrb_d = nc.dram_tensor("ones_rowb", [1, P], BF16, kind="ExternalInput")
    y_d = nc.dram_tensor("y", [NB, N, D], BF16, kind="ExternalOutput")

    x_r = x_d.ap().rearrange("b (p t) d -> b p (t d)", p=P)
    y_r = y_d.ap().rearrange("b (p t) d -> b p (t d)", p=P)

    with tile.TileContext(nc) as tc, ExitStack() as ctx:
        consts = ctx.enter_context(tc.tile_pool(name="consts", bufs=1))
        # persistent per-slab tiles (live across phases): 8 x tiles + 8 mu reps
        resid = ctx.enter_context(tc.tile_pool(name="resid", bufs=1))
        # batched [P, 256] working set
        bat = ctx.enter_context(tc.tile_pool(name="bat", bufs=1))
        # rotating big scratch
        hsp = ctx.enter_context(tc.tile_pool(name="hsp", bufs=1))
        esp = ctx.enter_context(tc.tile_pool(name="esp", bufs=2))
        osp = ctx.enter_context(tc.tile_pool(name="osp", bufs=2))
        tfp = ctx.enter_context(tc.tile_pool(name="tfp", bufs=1))
        sm = ctx.enter_context(tc.tile_pool(name="sm", bufs=1))
        psS = ctx.enter_context(tc.tile_pool(name="psS", bufs=2, space="PSUM"))
        psM = ctx.enter_context(tc.tile_pool(name="psM", bufs=1, space="PSUM"))
        psR = ctx.enter_context(tc.tile_pool(name="psR", bufs=1, space="PSUM"))

        # ---- constants ----
        wgt = consts.tile([1, 1], F32)
        nc.sync.dma_start(wgt[:], w_d.ap())
        onc = consts.tile([P, 1], BF16)
        nc.sync.dma_start(onc[:], onc_d.ap())
        oncf = consts.tile([P, 1], F32)
        nc.sync.dma_start(oncf[:], oncf_d.ap())
        onr = consts.tile([1, P], F32)
        nc.sync.dma_start(onr[:], onr_d.ap())
        onrb = consts.tile([1, P], BF16)
        nc.sync.dma_start(onrb[:], onrb_d.ap())

        # ---- persistent tiles ----
        xall = resid.tile([P, NB * TD], BF16, tag="xall")
        xb_t = [xall[:, b * TD : (b + 1) * TD] for b in range(NB)]
        mur_all = resid.tile([P, NB * D], BF16, tag="mur_all")
        mur_t = [mur_all[:, b * D : (b + 1) * D] for b in range(NB)]
        s_row = resid.tile([1, NB * D], F32, tag="s_row")   # block b = slab-b sum
        s_r3 = s_row[:].rearrange("q (b d) -> q b d", d=D)
        # batched [P, 256] tiles (column block b = slab b)
        x0_all = bat.tile([P, NB * T], F32, tag="x0")
        pd_all = bat.tile([P, NB * T], F32, tag="pd")
        al_all = bat.tile([P, NB * T], F32, tag="al")

        # =========== phase A: load + slab sums (PE/DMA only) ===========
        def gfold(dstap, dst_w, srcap, src_w):
            # halves-fold each slab block of [P, G*src_w] -> [P, G*dst_w]
            s3 = srcap.rearrange("p (b w) -> p b w", w=src_w)
            nc.vector.tensor_add(
                dstap.rearrange("p (b w) -> p b w", w=dst_w),
                s3[:, :, 0:dst_w], s3[:, :, dst_w:src_w],
            )

        for g in range(0, NB, 2):  # 2-slab chunks; one 3-level fold per chunk
            nc.sync.dma_start(xall[:, g * TD : (g + 1) * TD], x_r[g])
            nc.sync.dma_start(xall[:, (g + 1) * TD : (g + 2) * TD], x_r[g + 1])
            tf1 = tfp.tile([P, TD], BF16, tag="tf1")
            gfold(tf1[:], TD // 2, xall[:, g * TD : (g + 2) * TD], TD)
            tf2 = tfp.tile([P, TD // 2], BF16, tag="tf2")
            gfold(tf2[:], TD // 4, tf1[:], TD // 2)
            tf3 = tfp.tile([P, TD // 4], BF16, tag="tf3")
            gfold(tf3[:], TD // 8, tf2[:], TD // 4)
            for j in range(2):
                b = g + j
                s2 = psS.tile([1, 2 * D], F32, tag="s2")
                for c in range(2):
                    nc.tensor.matmul(
                        s2[:], onc[:],
                        tf3[:, (j * 2 + c) * 2 * D : (j * 2 + c + 1) * 2 * D],
                        start=(c == 0), stop=(c == 1),
                    )
                s2s = sm.tile([1, 2 * D], F32, tag="s2s", bufs=2)
                nc.scalar.copy(s2s[:], s2[:])
                nc.vector.tensor_add(
                    s_row[0:1, b * D : (b + 1) * D], s2s[0:1, 0:D],
                    s2s[0:1, D : 2 * D],
                )

        # =========== batched mu chain on partition-0 rows ===========
        p2 = sm.tile([1, NB * D], F32, tag="p2")
        nc.scalar.square(p2[:], s_row[:])
        ss8 = sm.tile([1, NB], F32, tag="ss8")
        nc.vector.tensor_reduce(
            ss8[:], p2[:].rearrange("q (b d) -> q b d", d=D),
            axis=mybir.AxisListType.X, op=OP.add,
        )
        s0r = sm.tile([1, NB], F32, tag="s0r")
        nc.vector.tensor_copy(s0r[:], s_r3[:, :, 0])
        s0sq = sm.tile([1, NB], F32, tag="s0sq")
        nc.scalar.square(s0sq[:], s0r[:])
        nls = sm.tile([1, NB], F32, tag="nls")
        nc.vector.scalar_tensor_tensor(
            out=nls[:], in0=s0sq[:], scalar=2.0, in1=ss8[:],
            op0=OP.mult, op1=OP.subtract,
        )
        nc.vector.tensor_scalar_max(nls[:], nls[:], EPS)
        lg8 = sm.tile([1, NB], F32, tag="lg8")
        nc.scalar.activation(lg8[:], nls[:], AF.Ln)
        rinv = sm.tile([1, NB], F32, tag="rinv")
        nc.scalar.activation(rinv[:], lg8[:], AF.Exp, scale=-0.5)
        mu_row = sm.tile([1, NB * D], F32, tag="mu_row")
        nc.vector.tensor_tensor(
            mu_row[:].rearrange("q (b d) -> q b d", d=D), s_r3,
            rinv[:].unsqueeze(2).broadcast_to([1, NB, D]), OP.mult,
        )
        mu0 = sm.tile([1, NB], F32, tag="mu0")
        nc.vector.tensor_mul(mu0[:], s0r[:], rinv[:])
        m2row = sm.tile([1, NB], F32, tag="m2row")
        nc.vector.tensor_scalar_mul(m2row[:], mu0[:], 2.0)
        srowE = sm.tile([1, 2 * NB], F32, tag="srowE")
        nc.vector.tensor_scalar_mul(srowE[:, 0:NB], mu0[:], float(bm0))  # -gamma
        denrow = sm.tile([1, NB], F32, tag="denrow")
        nc.vector.tensor_scalar(
            denrow[:], mu0[:], float(bm0), 1.0, op0=OP.mult, op1=OP.add
        )
        nc.vector.reciprocal(srowE[:, NB : 2 * NB], denrow[:])  # 1/(1-gamma)

        # replicate the mu-derived pair now -- hides under phase B
        W = NB * T
        browE = sm.tile([1, 2 * W], BF16, tag="browE")
        nc.vector.tensor_copy(
            browE[:].rearrange("q (s t) -> q s t", t=T),
            srowE[:].unsqueeze(2).broadcast_to([1, 2 * NB, T]),
        )
        blkE_ps = psR.tile([P, 2 * W], F32, tag="blkEps")
        nc.tensor.matmul(blkE_ps[:], onrb[:], browE[:], start=True, stop=True)
        blkE = bat.tile([P, 2 * W], F32, tag="blkE")
        nc.scalar.copy(blkE[:], blkE_ps[:])
        gnegb = blkE[:, 0:W]
        invdb = blkE[:, W : 2 * W]

        # [P, 256] block tensor of 2*mu0 for the batched alpha op
        m2brow = sm.tile([1, NB * T], F32, tag="m2brow")
        nc.vector.tensor_copy(
            m2brow[:].rearrange("q (b t) -> q b t", t=T),
            m2row[:].unsqueeze(2).broadcast_to([1, NB, T]),
        )
        m2_ps = psR.tile([P, NB * T], F32, tag="m2ps")
        nc.tensor.matmul(m2_ps[:], onr[:], m2brow[:], start=True, stop=True)
        m2blk = consts.tile([P, NB * T], F32)
        nc.scalar.copy(m2blk[:], m2_ps[:])

        # mu replicated across partitions, cast bf16: three <=512-wide matmuls
        for third in range(3):
            cw = NB * D // 3  # 344
            mp = psM.tile([P, cw], F32, tag="mp")
            nc.tensor.matmul(
                mp[:], onr[:], mu_row[0:1, third * cw : (third + 1) * cw],
                start=True, stop=True,
            )
            nc.scalar.copy(mur_all[:, third * cw : (third + 1) * cw], mp[:])

        # =========== phase B: per-slab alpha ===========
        for b in range(NB):
            bs = slice(b * T, (b + 1) * T)
            xb3 = xb_t[b].rearrange("p (t d) -> p t d", d=D)
            h = hsp.tile([P, TD], BF16, tag="h")
            h3 = h[:].rearrange("p (t d) -> p t d", d=D)
            mu_b = mur_t[b].unsqueeze(1).broadcast_to([P, T, D])
            nc.vector.tensor_tensor(h3, xb3, mu_b, OP.mult)  # 2x packed
            nc.vector.tensor_reduce(
                pd_all[:, bs], h3, axis=mybir.AxisListType.X, op=OP.add
            )

        # =========== batched coefficient pipeline on [P, 256] ===========

        def bt(tag):
            return bat.tile([P, W], F32, tag=tag, name=tag)

        # x0 for all slabs in one strided copy: [P, (b t d)] -> [P, (b t)]
        nc.vector.tensor_copy(
            x0_all[:].rearrange("p (b t) -> p b t", t=T),
            xall[:].rearrange("p (b t d) -> p b t d", t=T, d=D)[:, :, :, 0],
        )
        nc.vector.tensor_mul(al_all[:], x0_all[:], m2blk[:])
        nc.vector.tensor_sub(al_all[:], al_all[:], pd_all[:])
        nc.vector.tensor_scalar_max(al_all[:], al_all[:], 1.0 + EPS)
        sq = bt("sq")
        nc.scalar.square(sq[:], al_all[:])
        am1 = bt("am1")
        nc.vector.tensor_scalar(am1[:], sq[:], -1.0, EPS, op0=OP.add, op1=OP.max)
        lnA = bt("lnA")
        nc.scalar.activation(lnA[:], am1[:], AF.Ln)
        nu = bt("nu")
        nc.scalar.activation(nu[:], lnA[:], AF.Exp, scale=0.5)
        dsum = bt("dsum")
        nc.vector.tensor_add(dsum[:], al_all[:], nu[:])
        dd = bt("dd")
        nc.scalar.activation(dd[:], dsum[:], AF.Ln)
        rnu = bt("rnu")
        nc.vector.reciprocal(rnu[:], nu[:])
        c1 = bt("c1")
        nc.vector.tensor_mul(c1[:], dd[:], rnu[:])

        # var per slab: d^2 accumulated over each 32-col block, then col-sum
        ds1 = sm.tile([P, NB], F32, tag="ds1")
        scrW = bt("scrW")
        for b in range(NB):
            bs = slice(b * T, (b + 1) * T)
            nc.scalar.activation(
                scrW[:, bs], dd[:, bs], AF.Square, accum_out=ds1[:, b : b + 1]
            )
        var_ps = psR.tile([1, NB], F32, tag="var")
        nc.tensor.matmul(var_ps[:], oncf[:], ds1[:], start=True, stop=True)
        varm = sm.tile([1, NB], F32, tag="varm")
        nc.scalar.activation(varm[:], var_ps[:], AF.Copy, bias=1e-6, scale=1.0 / float(N))
        rv = sm.tile([1, NB], F32, tag="rv")
        nc.vector.reciprocal(rv[:], varm[:])
        w2sq = sm.tile([1, NB], F32, tag="w2sq")
        nc.vector.tensor_scalar_mul(w2sq[:], rv[:], wgt[:])
        lw = sm.tile([1, NB], F32, tag="lw")
        nc.scalar.activation(lw[:], w2sq[:], AF.Ln)
        srowL = sm.tile([1, 2 * NB], F32, tag="srowL")
        nc.scalar.activation(srowL[:, 0:NB], lw[:], AF.Exp, scale=0.5)
        nc.vector.tensor_scalar_mul(srowL[:, NB : 2 * NB], srowL[:, 0:NB], 0.5)

        # replicate the var-derived pair {w2, w2h} (critical path: bf16 matmul)
        browL = sm.tile([1, 2 * W], BF16, tag="browL")
        nc.vector.tensor_copy(
            browL[:].rearrange("q (s t) -> q s t", t=T),
            srowL[:].unsqueeze(2).broadcast_to([1, 2 * NB, T]),
        )
        blkL_ps = psR.tile([P, 2 * W], F32, tag="blkLps")
        nc.tensor.matmul(blkL_ps[:], onrb[:], browL[:], start=True, stop=True)
        blkL = bat.tile([P, 2 * W], F32, tag="blkL")
        nc.scalar.copy(blkL[:], blkL_ps[:])
        w2b = blkL[:, 0:W]
        w2hb = blkL[:, W : 2 * W]

        nn = bt("nn")
        nc.vector.tensor_mul(nn[:], dd[:], w2b[:])
        nc.vector.tensor_scalar_max(nn[:], nn[:], SQRT_EPS)
        ee = bt("ee")
        nc.scalar.activation(ee[:], nn[:], AF.Exp)
        em = bt("em")
        nc.scalar.activation(em[:], nn[:], AF.Exp, scale=-1.0)
        rn = bt("rn")
        nc.vector.reciprocal(rn[:], nn[:])
        sh = bt("sh")
        nc.vector.tensor_sub(sh[:], ee[:], em[:])
        ch = bt("ch")
        nc.vector.tensor_add(ch[:], ee[:], em[:])
        sc = bt("sc")
        nc.vector.tensor_mul(sc[:], sh[:], rn[:])       # 2*sinh(n)/n
        scW = bt("scW")
        nc.vector.tensor_mul(scW[:], sc[:], w2hb[:])    # sc*w2/2

        bet = bt("bet")
        nc.vector.tensor_scalar_mul(bet[:], x0_all[:], float(-bm0))
        t1 = bt("t1")
        nc.vector.tensor_mul(t1[:], al_all[:], gnegb[:])
        nc.vector.tensor_add(t1[:], t1[:], bet[:])
        k1 = bt("k1")
        nc.vector.tensor_mul(k1[:], t1[:], invdb[:])
        kf = bt("kf")
        nc.vector.tensor_mul(kf[:], k1[:], c1[:])

        Af = bt("Af")
        nc.vector.tensor_mul(Af[:], scW[:], c1[:])
        Cpre = bt("Cpre")
        nc.vector.tensor_mul(Cpre[:], scW[:], kf[:])
        # B = scW*(kf - c1*alpha) = Cpre - Af*alpha
        Bf = bt("Bf")
        nc.vector.tensor_mul(Bf[:], Af[:], al_all[:])
        nc.vector.tensor_sub(Bf[:], Cpre[:], Bf[:])
        Cf = bt("Cf")
        nc.vector.scalar_tensor_tensor(
            out=Cf[:], in0=ch[:], scalar=0.5, in1=Cpre[:], op0=OP.mult, op1=OP.add
        )
        Ab = bat.tile([P, W], BF16, tag="Ab")
        nc.vector.tensor_copy(Ab[:], Af[:])
        Bb = bat.tile([P, W], BF16, tag="Bb")
        nc.vector.tensor_copy(Bb[:], Bf[:])
        Cb = bat.tile([P, W], BF16, tag="Cb")
        nc.vector.tensor_copy(Cb[:], Cf[:])

        # =========== phase C: per-slab combine + store ===========
        for b in range(NB):
            bs = slice(b * T, (b + 1) * T)
            xb3 = xb_t[b].rearrange("p (t d) -> p t d", d=D)
            # dense bf16 expansions of the A/B columns (ACT, off DVE)
            aexp = esp.tile([P, TD], BF16, tag="aexp")
            a3 = aexp[:].rearrange("p (t d) -> p t d", d=D)
            nc.scalar.copy(a3, Ab[:, bs].unsqueeze(2).broadcast_to([P, T, D]))
            bexp = esp.tile([P, TD], BF16, tag="bexp")
            b3 = bexp[:].rearrange("p (t d) -> p t d", d=D)
            nc.gpsimd.tensor_copy(b3, Bb[:, bs].unsqueeze(2).broadcast_to([P, T, D]))

            r = hsp.tile([P, TD], BF16, tag="r")
            r3 = r[:].rearrange("p (t d) -> p t d", d=D)
            mu_b = mur_t[b].unsqueeze(1).broadcast_to([P, T, D])
            nc.vector.tensor_tensor(r3, b3, mu_b, OP.mult)       # 2x
            out_sb = osp.tile([P, TD], BF16, tag="o")
            o3 = out_sb[:].rearrange("p (t d) -> p t d", d=D)
            nc.vector.tensor_tensor(o3, xb3, a3, OP.mult)        # 2x
            nc.vector.tensor_add(out_sb[:], out_sb[:], r[:])     # 2x
            nc.vector.scalar_tensor_tensor(
                out=o3[:, :, 0], in0=Cb[:, bs], scalar=float(bm0), in1=o3[:, :, 0],
                op0=OP.mult, op1=OP.add,
            )
            nc.sync.dma_start(y_r[b], out_sb[:])

    real_tables = bacc.get_activation_tables
    bacc.get_activation_tables = _only_nat_log_exp_tables
    try:
        nc.compile()
    finally:
        bacc.get_activation_tables = real_tables
    return nc


# ---------------------------------------------------------------------------
# v1 builder (general-bias fallback) -- identical to the original baseline.
# ---------------------------------------------------------------------------

def build_kernel_v1(n_batch: int, has_bias: bool, bm0: float):
    nc = bacc.Bacc("TRN2", target_bir_lowering=False, debug=False)

    x_d = nc.dram_tensor("x", [n_batch, N, D], F32, kind="ExternalInput")
    bm_d = nc.dram_tensor("bm", [1, D], F32, kind="ExternalInput")
    bmt_d = nc.dram_tensor("bmt", [1, D], F32, kind="ExternalInput")
    w_d = nc.dram_tensor("w", [1, 1], F32, kind="ExternalInput")
    onc_d = nc.dram_tensor("ones_col", [P, 1], F32, kind="ExternalInput")
    onr_d = nc.dram_tensor("ones_row", [1, P], F32, kind="ExternalInput")
    onrb_d = nc.dram_tensor("ones_rowb", [1, P], BF16, kind="ExternalInput")
    idn_d = nc.dram_tensor("ident", [P, P], F32, kind="ExternalInput")
    y_d = nc.dram_tensor("y", [n_batch, N, D], F32, kind="ExternalOutput")

    x_r = x_d.ap().rearrange("b (p t) d -> b p (t d)", p=P)
    y_r = y_d.ap().rearrange("b (p t) d -> b p (t d)", p=P)

    with tile.TileContext(nc) as tc, ExitStack() as ctx:
        consts = ctx.enter_context(tc.tile_pool(name="consts", bufs=1))
        xpool = ctx.enter_context(tc.tile_pool(name="xp", bufs=2))
        opool = ctx.enter_context(tc.tile_pool(name="op", bufs=3))
        mpool = ctx.enter_context(tc.tile_pool(name="mp", bufs=2))
        pp = ctx.enter_context(tc.tile_pool(name="pp", bufs=3))
        sm = ctx.enter_context(tc.tile_pool(name="sm", bufs=3))
        btp = ctx.enter_context(tc.tile_pool(name="btp", bufs=2))
        psA = ctx.enter_context(tc.tile_pool(name="psA", bufs=3, space="PSUM"))

        bm = consts.tile([1, D], F32)
        nc.sync.dma_start(bm[:], bm_d.ap())
        bmt = consts.tile([1, D], F32)
        nc.sync.dma_start(bmt[:], bmt_d.ap())
        wgt = consts.tile([1, 1], F32)
        nc.sync.dma_start(wgt[:], w_d.ap())
        onc = consts.tile([P, 1], F32)
        nc.sync.dma_start(onc[:], onc_d.ap())
        onr = consts.tile([1, P], F32)
        nc.sync.dma_start(onr[:], onr_d.ap())
        onrb = consts.tile([1, P], BF16)
        nc.sync.dma_start(onrb[:], onrb_d.ap())
        idn = consts.tile([P, P], F32)
        nc.sync.dma_start(idn[:], idn_d.ap())

        if has_bias:
            bmt_ps = psA.tile([P, D], F32, tag="ps_small")
            nc.tensor.matmul(bmt_ps[:], onr[:], bmt[:], start=True, stop=True)
            bmt_rep = consts.tile([P, D], F32)
            nc.scalar.copy(bmt_rep[:], bmt_ps[:])
            bm_ps = psA.tile([P, D], F32, tag="ps_small")
            nc.tensor.matmul(bm_ps[:], onr[:], bm[:], start=True, stop=True)
            bm_rep = consts.tile([P, D], F32)
            nc.scalar.copy(bm_rep[:], bm_ps[:])

        def stage1(b):
            st = {}
            xb = xpool.tile([P, T * D], F32)
            nc.sync.dma_start(xb[:], x_r[b])
            xb3 = xb[:].rearrange("p (t d) -> p t d", d=D)
            st["xb3"] = xb3
            out_sb = opool.tile([P, T * D], F32)
            st["out_sb"] = out_sb
            h13 = out_sb[:].rearrange("p (t d) -> p t d", d=D)

            s_ps = psA.tile([1, D], F32, tag="ps_small")
            for t in range(T):
                nc.tensor.matmul(
                    s_ps[:], onc[:], xb3[:, t, :], start=(t == 0), stop=(t == T - 1)
                )
            s_sb = sm.tile([1, D], F32)
            nc.scalar.copy(s_sb[:], s_ps[:])

            scr_d = sm.tile([1, D], F32)
            ssum = sm.tile([1, 1], F32)
            nc.vector.tensor_mul(scr_d[:], s_sb[:], s_sb[:])
            nc.vector.tensor_reduce(
                ssum[:], scr_d[:], axis=mybir.AxisListType.X, op=OP.add
            )
            s0sq = sm.tile([1, 1], F32)
            nc.scalar.square(s0sq[:], s_sb[0:1, 0:1])
            nls = sm.tile([1, 1], F32)
            nc.vector.scalar_tensor_tensor(
                out=nls[:], in0=s0sq[:], scalar=2.0, in1=ssum[:],
                op0=OP.mult, op1=OP.subtract,
            )
            nc.vector.tensor_scalar_max(nls[:], nls[:], EPS)
            rls = sm.tile([1, 1], F32)
            nc.vector.reciprocal(rls[:], nls[:])
            rsq = sm.tile([1, 1], F32)
            nc.scalar.sqrt(rsq[:], rls[:])
            mu = sm.tile([1, D], F32)
            nc.vector.tensor_scalar_mul(mu[:], s_sb[:], rsq[:])

            mu_ps = psA.tile([P, D], F32, tag="ps_small")
            nc.tensor.matmul(mu_ps[:], onr[:], mu[:], start=True, stop=True)
            mu_rep = mpool.tile([P, D], F32)
            nc.scalar.copy(mu_rep[:], mu_ps[:])

            stageA = sm.tile([1, 3], F32)
            nc.scalar.mul(stageA[:, 0:1], mu[0:1, 0:1], 2.0)
            scr_d2 = sm.tile([1, D], F32)
            nc.vector.tensor_mul(scr_d2[:], mu[:], bmt[:])
            g_pos = sm.tile([1, 1], F32)
            nc.vector.tensor_reduce(
                g_pos[:], scr_d2[:], axis=mybir.AxisListType.X, op=OP.add
            )
            nc.scalar.mul(stageA[:, 1:2], g_pos[:], -1.0)
            one_mg = sm.tile([1, 1], F32)
            nc.scalar.activation(one_mg[:], g_pos[:], AF.Identity, scale=-1.0, bias=1.0)
            nc.vector.reciprocal(stageA[:, 2:3], one_mg[:])
            repsA_ps = psA.tile([P, 3], F32, tag="ps_small")
            nc.tensor.matmul(repsA_ps[:], onr[:], stageA[:], start=True, stop=True)
            repsA = pp.tile([P, 3], F32)
            nc.scalar.copy(repsA[:], repsA_ps[:])
            mu0x2_rep = repsA[:, 0:1]
            ngam_rep = repsA[:, 1:2]
            invden_rep = repsA[:, 2:3]

            mu_b = mu_rep[:].unsqueeze(1).broadcast_to([P, T, D])
            nc.vector.tensor_tensor(h13, xb3, mu_b, OP.mult)

            pdot = pp.tile([P, T], F32)
            scrA = sm.tile([P, D], F32)
            nc.vector.tensor_reduce(
                pdot[:], h13, axis=mybir.AxisListType.X, op=OP.add
            )

            x0t = pp.tile([P, T], F32)
            nc.scalar.copy(x0t[:], xb3[:, :, 0])
            alpha = pp.tile([P, T], F32)
            nc.vector.scalar_tensor_tensor(
                out=alpha[:], in0=x0t[:], scalar=mu0x2_rep, in1=pdot[:],
                op0=OP.mult, op1=OP.subtract,
            )
            nc.vector.tensor_scalar_max(alpha[:], alpha[:], 1.0 + EPS)

            sq = pp.tile([P, T], F32)
            nc.scalar.square(sq[:], alpha[:])
            am1 = pp.tile([P, T], F32)
            nc.vector.tensor_scalar_add(am1[:], sq[:], -1.0)
            nc.vector.tensor_scalar_max(am1[:], am1[:], EPS)
            nu = pp.tile([P, T], F32)
            nc.scalar.sqrt(nu[:], am1[:])
            dsum = pp.tile([P, T], F32)
            nc.vector.tensor_add(dsum[:], alpha[:], nu[:])
            dd = pp.tile([P, T], F32)
            nc.scalar.activation(dd[:], dsum[:], AF.Ln)
            rnu = pp.tile([P, T], F32)
            nc.vector.reciprocal(rnu[:], nu[:])
            c1 = pp.tile([P, T], F32)
            nc.vector.tensor_mul(c1[:], dd[:], rnu[:])

            scrT = pp.tile([P, T], F32)
            ds1 = pp.tile([P, 1], F32)
            nc.scalar.activation(scrT[:], dd[:], AF.Square, accum_out=ds1[:])
            var_ps = psA.tile([1, 1], F32, tag="ps_small")
            nc.tensor.matmul(var_ps[:], onc[:], ds1[:], start=True, stop=True)
            varm = sm.tile([1, 1], F32)
            nc.scalar.activation(
                varm[:], var_ps[:], AF.Copy, bias=1e-6, scale=1.0 / float(N)
            )
            rv = sm.tile([1, 1], F32)
            nc.vector.reciprocal(rv[:], varm[:])
            w2sq = sm.tile([1, 1], F32)
            nc.vector.tensor_mul(w2sq[:], rv[:], wgt[:])
            stageB = sm.tile([1, 2], F32)
            nc.scalar.sqrt(stageB[:, 0:1], w2sq[:])
            nc.scalar.mul(stageB[:, 1:2], stageB[:, 0:1], 0.5)
            repsB_ps = psA.tile([P, 2], F32, tag="ps_small")
            nc.tensor.matmul(repsB_ps[:], onr[:], stageB[:], start=True, stop=True)
            repsB = pp.tile([P, 2], F32)
            nc.scalar.copy(repsB[:], repsB_ps[:])
            w2_rep = repsB[:, 0:1]
            w2h_rep = repsB[:, 1:2]

            bet = pp.tile([P, T], F32)
            if has_bias:
                hb = btp.tile([P, T * D], F32, tag="hb")
                hb3 = hb[:].rearrange("p (t d) -> p t d", d=D)
                bmt_b = bmt_rep[:].unsqueeze(1).broadcast_to([P, T, D])
                nc.vector.tensor_tensor(hb3, xb3, bmt_b, OP.mult)
                for t in range(T):
                    nc.scalar.activation(
                        scrA[:], hb3[:, t, :], AF.Copy, accum_out=bet[:, t : t + 1]
                    )
            else:
                nc.vector.tensor_scalar_mul(bet[:], x0t[:], float(-bm0))

            t1 = pp.tile([P, T], F32)
            nc.vector.scalar_tensor_tensor(
                out=t1[:], in0=alpha[:], scalar=ngam_rep, in1=bet[:],
                op0=OP.mult, op1=OP.add,
            )
            k1 = pp.tile([P, T], F32)
            nc.vector.tensor_scalar_mul(k1[:], t1[:], invden_rep)
            kf = pp.tile([P, T], F32)
            nc.vector.tensor_mul(kf[:], k1[:], c1[:])

            nn = pp.tile([P, T], F32)
            nc.vector.tensor_scalar_mul(nn[:], dd[:], w2_rep)
            nc.vector.tensor_scalar_max(nn[:], nn[:], SQRT_EPS)
            ee = pp.tile([P, T], F32)
            nc.scalar.activation(ee[:], nn[:], AF.Exp)
            em = pp.tile([P, T], F32)
            nc.scalar.activation(em[:], nn[:], AF.Exp, scale=-1.0)
            rn = pp.tile([P, T], F32)
            nc.vector.reciprocal(rn[:], nn[:])
            sh = pp.tile([P, T], F32)
            nc.vector.tensor_sub(sh[:], ee[:], em[:])
            sc = pp.tile([P, T], F32)
            nc.vector.tensor_mul(sc[:], sh[:], rn[:])
            ch = pp.tile([P, T], F32)
            nc.vector.tensor_add(ch[:], ee[:], em[:])

            Aco = pp.tile([P, T], F32)
            a3 = pp.tile([P, T], F32)
            nc.vector.tensor_scalar_mul(a3[:], c1[:], w2h_rep)
            nc.vector.tensor_mul(Aco[:], sc[:], a3[:])
            st["Aco"] = Aco

            ca = pp.tile([P, T], F32)
            nc.vector.tensor_mul(ca[:], c1[:], alpha[:])
            kc = pp.tile([P, T], F32)
            nc.vector.tensor_sub(kc[:], kf[:], ca[:])
            b3 = pp.tile([P, T], F32)
            nc.vector.tensor_scalar_mul(b3[:], kc[:], w2h_rep)
            Bco = pp.tile([P, T], F32)
            nc.vector.tensor_mul(Bco[:], sc[:], b3[:])
            c3 = pp.tile([P, T], F32)
            nc.vector.tensor_scalar_mul(c3[:], kf[:], w2h_rep)
            c0 = pp.tile([P, T], F32)
            nc.vector.tensor_mul(c0[:], sc[:], c3[:])
            Cco = pp.tile([P, T], F32)
            nc.vector.scalar_tensor_tensor(
                out=Cco[:], in0=ch[:], scalar=0.5, in1=c0[:],
                op0=OP.mult, op1=OP.add,
            )
            st["Bco"] = Bco
            st["Cco"] = Cco
            st["mu_rep"] = mu_rep
            st["b"] = b
            return st

        def stage2(st):
            xb3 = st["xb3"]
            o3 = st["out_sb"][:].rearrange("p (t d) -> p t d", d=D)
            mu_rep, Aco, Bco, Cco, b = st["mu_rep"], st["Aco"], st["Bco"], st["Cco"], st["b"]
            rr = xpool.tile([P, T * D], F32, tag="rr")
            r3 = rr[:].rearrange("p (t d) -> p t d", d=D)
            A_b = Aco[:].unsqueeze(2).broadcast_to([P, T, D])
            B_b = Bco[:].unsqueeze(2).broadcast_to([P, T, D])
            mu_b2 = mu_rep[:].unsqueeze(1).broadcast_to([P, T, D])
            nc.vector.tensor_tensor(r3, B_b, mu_b2, OP.mult)
            nc.vector.tensor_tensor(o3, xb3, A_b, OP.mult)
            nc.vector.tensor_tensor(o3, o3, r3, OP.add)
            if has_bias:
                C_b = Cco[:].unsqueeze(2).broadcast_to([P, T, D])
                bm_b = bm_rep[:].unsqueeze(1).broadcast_to([P, T, D])
                nc.vector.tensor_tensor(r3, C_b, bm_b, OP.mult)
                nc.vector.tensor_tensor(o3, o3, r3, OP.add)
            else:
                nc.vector.scalar_tensor_tensor(
                    out=o3[:, :, 0], in0=Cco[:], scalar=float(bm0), in1=o3[:, :, 0],
                    op0=OP.mult, op1=OP.add,
                )
            nc.sync.dma_start(y_r[b], st["out_sb"][:])

        prev = None
        for b in range(n_batch):
            cur = stage1(b)
            if prev is not None:
                stage2(prev)
            prev = cur
        stage2(prev)

    nc.compile()
    return nc


def _host_bias_manifold(bias: np.ndarray):
    b32 = np.asarray(bias, dtype=np.float32)
    sq = np.float32(np.sum(b32 * b32, dtype=np.float32))
    nrm2 = np.maximum(sq, np.float32(EPS))
    n = np.sqrt(nrm2)
    bm = np.zeros(D, dtype=np.float32)
    bm[0] = np.cosh(n)
    bm[1:] = (np.sinh(n) / n) * b32
    return bm


_CACHE = {}


def _get_nc_v2(bm0):
    if "v2" not in _CACHE:
        _CACHE["v2"] = build_kernel_v2(bm0)
    return _CACHE["v2"]


def _get_nc_v1(n_batch, has_bias, bm0):
    key = ("v1", n_batch, has_bias)
    if key not in _CACHE:
        _CACHE[key] = build_kernel_v1(n_batch, has_bias, bm0)
    return _CACHE[key]


def _make_in_maps_v2(x, weight):
    xb = x.astype(ml_dtypes.bfloat16)
    b_sh = x.shape[0] // N_CORES
    common = {
        "w": np.asarray(weight, dtype=np.float32).reshape(1, 1),
        "ones_col": np.ones((P, 1), dtype=ml_dtypes.bfloat16),
        "ones_colf": np.ones((P, 1), dtype=np.float32),
        "ones_row": np.ones((1, P), dtype=np.float32),
        "ones_rowb": np.ones((1, P), dtype=ml_dtypes.bfloat16),
    }
    return [
        {"x": np.ascontiguousarray(xb[c * b_sh : (c + 1) * b_sh]), **common}
        for c in range(N_CORES)
    ]


def _make_in_maps_v1(x, bias, weight):
    bias = np.asarray(bias, dtype=np.float32)
    bm = _host_bias_manifold(bias)
    bmt = bm.copy()
    bmt[0] = -bmt[0]
    b_sh = x.shape[0] // N_CORES
    common = {
        "bm": bm.reshape(1, D),
        "bmt": bmt.reshape(1, D),
        "w": np.asarray(weight, dtype=np.float32).reshape(1, 1),
        "ones_col": np.ones((P, 1), dtype=np.float32),
        "ones_row": np.ones((1, P), dtype=np.float32),
        "ident": np.eye(P, dtype=np.float32),
    }
    return [
        {"x": np.ascontiguousarray(x[c * b_sh : (c + 1) * b_sh]), **common}
        for c in range(N_CORES)
    ]


def kernel(x, bias, weight):
    from concourse.bass_utils import run_bass_kernel_spmd

    x = np.ascontiguousarray(np.asarray(x, dtype=np.float32))
    assert x.shape == (B_FULL, N, D), x.shape
    bias = np.asarray(bias, dtype=np.float32)
    has_bias = bool(np.any(bias != 0))
    bm = _host_bias_manifold(bias)
    if has_bias:
        in_maps = _make_in_maps_v1(x, bias, weight)
        nc = _get_nc_v1(B_FULL // N_CORES, True, float(bm[0]))
    else:
        in_maps = _make_in_maps_v2(x, weight)
        nc = _get_nc_v2(float(bm[0]))
    res = run_bass_kernel_spmd(nc, in_maps, core_ids=list(range(N_CORES)))
    y = np.concatenate(
        [res.results[c]["y"].astype(np.float32) for c in range(N_CORES)], axis=0
    )
    return y


# revision 26
# speedup vs baseline: 1.0405x; 1.0168x over previous
"""Trainium2 Bass kernel for Lorentz (hyperboloid) batch norm.

Full-input contract: kernel(**inputs) takes x [64,4096,129] f32, bias [128],
weight scalar; returns y [64,4096,129] f32.  Internally shards batch dim
across 8 NeuronCores (8 batches/core) and runs one Bass/Tile kernel SPMD.

v2 design notes (no-bias fast path; bias!=0 falls back to the v1 builder):
  - x is cast to bf16 on host; all big streams are bf16 (rel-err budget 2e-2,
    measured ~3e-3 end to end).  Stats/coefficients stay f32.
  - per-batch big ops only; all small per-point algebra is batched across the
    8 slabs into [128, 256] tiles (column block b = slab b) to cut
    instruction count (~930 -> ~280) since HW per-instruction overhead
    dominates the measured time.
  - ACT uses only {ln, exp, square, copy, identity} = one table set, zero
    table reloads (sqrt is exp(0.5*ln)).
  - A/B coefficient columns are pre-expanded to dense [128, T*D] bf16 rows on
    ACT so every big DVE multiply runs in 2x_1p packed mode.

Math per slab (reductions over N=4096 points):
  s     = sum_i x_i ;  mu = s / sqrt(max(2*s0^2 - <s,s>, EPS))
  alpha_i = 2*mu0*x_i0 - <mu, x_i>  (clamped >= 1+EPS)
  nu = sqrt(alpha^2-1) ; d = ln(alpha+nu) ; c1 = d/nu
  var = mean(d^2) ; w2 = sqrt(weight/(var+1e-6))
  gamma = -bm0*mu0 ; bet_i = -bm0*x_i0 ; k = c1*(bet - alpha*gamma)/(1-gamma)
  n = max(w2*d, sqrt(EPS)) ; sc = 2*sinh(n)/n ; ch = 2*cosh(n)
  A = sc*(w2/2)*c1 ; B = sc*(w2/2)*(k - c1*alpha) ; C = sc*(w2/2)*k + ch/2
  out_i = A_i*x_i + B_i*mu  (+ C_i*bm0 on component 0)
"""

import numpy as np
from contextlib import ExitStack

import ml_dtypes
import concourse.bacc as bacc
import concourse.tile as tile
from concourse import mybir

AF = mybir.ActivationFunctionType
OP = mybir.AluOpType
F32 = mybir.dt.float32
BF16 = mybir.dt.bfloat16

N_CORES = 8
B_FULL, N, D = 64, 4096, 129
P, T = 128, 32            # N = P*T points per batch; point (p,t) = p*T + t
TD = T * D                # 4128
NB = 8                    # batches per core
EPS = 1e-7
SQRT_EPS = float(np.sqrt(np.float32(EPS)))


def _only_nat_log_exp_tables(arch):
    """Activation tables with every set except natural_log_exp_and_others
    emptied: the ATL chooser assigns ln->natural_log and exp->exp_and_others
    (first containing set), reloading tables on every ln/exp alternation.
    All activation funcs this kernel uses (copy/identity/square/ln/exp) live
    in natural_log_exp_and_others, so forcing that set yields ONE table load.
    Set ids (dict order) are preserved so act_func_set_id stays valid."""
    from concourse.hw_specs import get_activation_tables as _real
    tabs = _real(arch)
    keep = "natural_log_exp_and_others"
    return {name: (s if name == keep else set()) for name, s in tabs.items()}


def build_kernel_v2(bm0: float):
    """No-bias fast path: one core processing NB slabs, x/y in bf16."""
    nc = bacc.Bacc("TRN2", target_bir_lowering=False, debug=False)

    x_d = nc.dram_tensor("x", [NB, N, D], BF16, kind="ExternalInput")
    w_d = nc.dram_tensor("w", [1, 1], F32, kind="ExternalInput")
    onc_d = nc.dram_tensor("ones_col", [P, 1], BF16, kind="ExternalInput")
    oncf_d = nc.dram_tensor("ones_colf", [P, 1], F32, kind="ExternalInput")
    onr_d = nc.dram_tensor("ones_row", [1, P], F32, kind="ExternalInput")
    onrb_d = nc.dram_tensor("ones_rowb", [1, P], BF16, kind="ExternalInput")
    y_d = nc.dram_tensor("y", [NB, N, D], BF16, kind="ExternalOutput")

    x_r = x_d.ap().rearrange("b (p t) d -> b p (t d)", p=P)
    y_r = y_d.ap().rearrange("b (p t) d -> b p (t d)", p=P)

    with tile.TileContext(nc) as tc, ExitStack() as ctx:
        consts = ctx.enter_context(tc.tile_pool(name="consts", bufs=1))
        # persistent per-slab tiles (live across phases): 8 x tiles + 8 mu reps
        resid = ctx.enter_context(tc.tile_pool(name="resid", bufs=1))
        # batched [P, 256] working set
        bat = ctx.enter_context(tc.tile_pool(name="bat", bufs=1))
        # rotating big scratch
        hsp = ctx.enter_context(tc.tile_pool(name="hsp", bufs=1))
        esp = ctx.enter_context(tc.tile_pool(name="esp", bufs=2))
        osp = ctx.enter_context(tc.tile_pool(name="osp", bufs=2))
        tfp = ctx.enter_context(tc.tile_pool(name="tfp", bufs=1))
        sm = ctx.enter_context(tc.tile_pool(name="sm", bufs=1))
        psS = ctx.enter_context(tc.tile_pool(name="psS", bufs=2, space="PSUM"))
        psM = ctx.enter_context(tc.tile_pool(name="psM", bufs=1, space="PSUM"))
        psR = ctx.enter_context(tc.tile_pool(name="psR", bufs=1, space="PSUM"))

        # ---- persistent x tile first: its pair-0 DMA must lead the queue ----
        xall = resid.tile([P, NB * TD], BF16, tag="xall")
        nc.sync.dma_start(xall[:, 0:TD], x_r[0])
        nc.sync.dma_start(xall[:, TD : 2 * TD], x_r[1])

        # ---- constants ----
        wgt = consts.tile([1, 1], F32)
        nc.sync.dma_start(wgt[:], w_d.ap())
        onc = consts.tile([P, 1], BF16)
        nc.sync.dma_start(onc[:], onc_d.ap())
        oncf = consts.tile([P, 1], F32)
        nc.sync.dma_start(oncf[:], oncf_d.ap())
        onr = consts.tile([1, P], F32)
        nc.sync.dma_start(onr[:], onr_d.ap())
        onrb = consts.tile([1, P], BF16)
        nc.sync.dma_start(onrb[:], onrb_d.ap())

        # ---- persistent tiles ----
        xb_t = [xall[:, b * TD : (b + 1) * TD] for b in range(NB)]
        mur_all = resid.tile([P, NB * D], BF16, tag="mur_all")
        mur_t = [mur_all[:, b * D : (b + 1) * D] for b in range(NB)]
        s_row = resid.tile([1, NB * D], F32, tag="s_row")   # block b = slab-b sum
        s_r3 = s_row[:].rearrange("q (b d) -> q b d", d=D)
        # batched [P, 256] tiles (column block b = slab b)
        x0_all = bat.tile([P, NB * T], F32, tag="x0")
        pd_all = bat.tile([P, NB * T], F32, tag="pd")
        al_all = bat.tile([P, NB * T], F32, tag="al")

        # =========== phase A: load + slab sums (PE/DMA only) ===========
        def gfold(dstap, dst_w, srcap, src_w):
            # halves-fold each slab block of [P, G*src_w] -> [P, G*dst_w]
            s3 = srcap.rearrange("p (b w) -> p b w", w=src_w)
            nc.vector.tensor_add(
                dstap.rearrange("p (b w) -> p b w", w=dst_w),
                s3[:, :, 0:dst_w], s3[:, :, dst_w:src_w],
            )

        for g in range(0, NB, 2):  # 2-slab chunks; one 3-level fold per chunk
            if g > 0:  # pair 0 was issued ahead of the constant loads
                nc.sync.dma_start(xall[:, g * TD : (g + 1) * TD], x_r[g])
                nc.sync.dma_start(xall[:, (g + 1) * TD : (g + 2) * TD], x_r[g + 1])
            tf1 = tfp.tile([P, TD], BF16, tag="tf1")
            gfold(tf1[:], TD // 2, xall[:, g * TD : (g + 2) * TD], TD)
            tf2 = tfp.tile([P, TD // 2], BF16, tag="tf2")
            gfold(tf2[:], TD // 4, tf1[:], TD // 2)
            tf3 = tfp.tile([P, TD // 4], BF16, tag="tf3")
            gfold(tf3[:], TD // 8, tf2[:], TD // 4)
            for j in range(2):
                b = g + j
                s2 = psS.tile([1, 2 * D], F32, tag="s2")
                for c in range(2):
                    nc.tensor.matmul(
                        s2[:], onc[:],
                        tf3[:, (j * 2 + c) * 2 * D : (j * 2 + c + 1) * 2 * D],
                        start=(c == 0), stop=(c == 1),
                    )
                s2s = sm.tile([1, 2 * D], F32, tag="s2s", bufs=2)
                nc.scalar.copy(s2s[:], s2[:])
                nc.vector.tensor_add(
                    s_row[0:1, b * D : (b + 1) * D], s2s[0:1, 0:D],
                    s2s[0:1, D : 2 * D],
                )

        # =========== batched mu chain on partition-0 rows ===========
        p2 = sm.tile([1, NB * D], F32, tag="p2")
        nc.scalar.square(p2[:], s_row[:])
        ss8 = sm.tile([1, NB], F32, tag="ss8")
        nc.vector.tensor_reduce(
            ss8[:], p2[:].rearrange("q (b d) -> q b d", d=D),
            axis=mybir.AxisListType.X, op=OP.add,
        )
        s0r = sm.tile([1, NB], F32, tag="s0r")
        nc.vector.tensor_copy(s0r[:], s_r3[:, :, 0])
        s0sq = sm.tile([1, NB], F32, tag="s0sq")
        nc.scalar.square(s0sq[:], s0r[:])
        nls = sm.tile([1, NB], F32, tag="nls")
        nc.vector.scalar_tensor_tensor(
            out=nls[:], in0=s0sq[:], scalar=2.0, in1=ss8[:],
            op0=OP.mult, op1=OP.subtract,
        )
        nc.vector.tensor_scalar_max(nls[:], nls[:], EPS)
        lg8 = sm.tile([1, NB], F32, tag="lg8")
        nc.scalar.activation(lg8[:], nls[:], AF.Ln)
        rinv = sm.tile([1, NB], F32, tag="rinv")
        nc.scalar.activation(rinv[:], lg8[:], AF.Exp, scale=-0.5)
        mu_row = sm.tile([1, NB * D], F32, tag="mu_row")
        nc.vector.tensor_tensor(
            mu_row[:].rearrange("q (b d) -> q b d", d=D), s_r3,
            rinv[:].unsqueeze(2).broadcast_to([1, NB, D]), OP.mult,
        )
        mu0 = sm.tile([1, NB], F32, tag="mu0")
        nc.vector.tensor_mul(mu0[:], s0r[:], rinv[:])
        m2row = sm.tile([1, NB], F32, tag="m2row")
        nc.vector.tensor_scalar_mul(m2row[:], mu0[:], 2.0)
        srowE = sm.tile([1, 2 * NB], F32, tag="srowE")
        nc.vector.tensor_scalar_mul(srowE[:, 0:NB], mu0[:], float(bm0))  # -gamma
        denrow = sm.tile([1, NB], F32, tag="denrow")
        nc.vector.tensor_scalar(
            denrow[:], mu0[:], float(bm0), 1.0, op0=OP.mult, op1=OP.add
        )
        nc.vector.reciprocal(srowE[:, NB : 2 * NB], denrow[:])  # 1/(1-gamma)

        # replicate the mu-derived pair now -- hides under phase B
        W = NB * T
        browE = sm.tile([1, 2 * W], BF16, tag="browE")
        nc.vector.tensor_copy(
            browE[:].rearrange("q (s t) -> q s t", t=T),
            srowE[:].unsqueeze(2).broadcast_to([1, 2 * NB, T]),
        )
        blkE_ps = psR.tile([P, 2 * W], F32, tag="blkEps")
        nc.tensor.matmul(blkE_ps[:], onrb[:], browE[:], start=True, stop=True)
        blkE = bat.tile([P, 2 * W], F32, tag="blkE")
        nc.scalar.copy(blkE[:], blkE_ps[:])
        gnegb = blkE[:, 0:W]
        invdb = blkE[:, W : 2 * W]

        # [P, 256] block tensor of 2*mu0 for the batched alpha op
        m2brow = sm.tile([1, NB * T], F32, tag="m2brow")
        nc.vector.tensor_copy(
            m2brow[:].rearrange("q (b t) -> q b t", t=T),
            m2row[:].unsqueeze(2).broadcast_to([1, NB, T]),
        )
        m2_ps = psR.tile([P, NB * T], F32, tag="m2ps")
        nc.tensor.matmul(m2_ps[:], onr[:], m2brow[:], start=True, stop=True)
        m2blk = consts.tile([P, NB * T], F32)
        nc.scalar.copy(m2blk[:], m2_ps[:])

        # mu replicated across partitions, cast bf16: three <=512-wide matmuls
        for third in range(3):
            cw = NB * D // 3  # 344
            mp = psM.tile([P, cw], F32, tag="mp")
            nc.tensor.matmul(
                mp[:], onr[:], mu_row[0:1, third * cw : (third + 1) * cw],
                start=True, stop=True,
            )
            nc.scalar.copy(mur_all[:, third * cw : (third + 1) * cw], mp[:])

        # =========== phase B: per-slab alpha ===========
        for b in range(NB):
            bs = slice(b * T, (b + 1) * T)
            xb3 = xb_t[b].rearrange("p (t d) -> p t d", d=D)
            h = hsp.tile([P, TD], BF16, tag="h")
            h3 = h[:].rearrange("p (t d) -> p t d", d=D)
            mu_b = mur_t[b].unsqueeze(1).broadcast_to([P, T, D])
            nc.vector.tensor_tensor(h3, xb3, mu_b, OP.mult)  # 2x packed
            nc.vector.tensor_reduce(
                pd_all[:, bs], h3, axis=mybir.AxisListType.X, op=OP.add
            )

        # =========== batched coefficient pipeline on [P, 256] ===========

        def bt(tag):
            return bat.tile([P, W], F32, tag=tag, name=tag)

        # x0 for all slabs in one strided copy: [P, (b t d)] -> [P, (b t)]
        nc.vector.tensor_copy(
            x0_all[:].rearrange("p (b t) -> p b t", t=T),
            xall[:].rearrange("p (b t d) -> p b t d", t=T, d=D)[:, :, :, 0],
        )
        nc.vector.tensor_mul(al_all[:], x0_all[:], m2blk[:])
        nc.vector.tensor_sub(al_all[:], al_all[:], pd_all[:])
        nc.vector.tensor_scalar_max(al_all[:], al_all[:], 1.0 + EPS)
        sq = bt("sq")
        nc.scalar.square(sq[:], al_all[:])
        am1 = bt("am1")
        nc.vector.tensor_scalar(am1[:], sq[:], -1.0, EPS, op0=OP.add, op1=OP.max)
        lnA = bt("lnA")
        nc.scalar.activation(lnA[:], am1[:], AF.Ln)
        nu = bt("nu")
        nc.scalar.activation(nu[:], lnA[:], AF.Exp, scale=0.5)
        dsum = bt("dsum")
        nc.vector.tensor_add(dsum[:], al_all[:], nu[:])
        dd = bt("dd")
        nc.scalar.activation(dd[:], dsum[:], AF.Ln)
        rnu = bt("rnu")
        nc.vector.reciprocal(rnu[:], nu[:])
        c1 = bt("c1")
        nc.vector.tensor_mul(c1[:], dd[:], rnu[:])

        # var per slab: d^2 accumulated over each 32-col block, then col-sum
        ds1 = sm.tile([P, NB], F32, tag="ds1")
        scrW = bt("scrW")
        for b in range(NB):
            bs = slice(b * T, (b + 1) * T)
            nc.scalar.activation(
                scrW[:, bs], dd[:, bs], AF.Square, accum_out=ds1[:, b : b + 1]
            )
        var_ps = psR.tile([1, NB], F32, tag="var")
        nc.tensor.matmul(var_ps[:], oncf[:], ds1[:], start=True, stop=True)
        varm = sm.tile([1, NB], F32, tag="varm")
        nc.scalar.activation(varm[:], var_ps[:], AF.Copy, bias=1e-6, scale=1.0 / float(N))
        rv = sm.tile([1, NB], F32, tag="rv")
        nc.vector.reciprocal(rv[:], varm[:])
        w2sq = sm.tile([1, NB], F32, tag="w2sq")
        nc.vector.tensor_scalar_mul(w2sq[:], rv[:], wgt[:])
        lw = sm.tile([1, NB], F32, tag="lw")
        nc.scalar.activation(lw[:], w2sq[:], AF.Ln)
        srowL = sm.tile([1, 2 * NB], F32, tag="srowL")
        nc.scalar.activation(srowL[:, 0:NB], lw[:], AF.Exp, scale=0.5)
        nc.vector.tensor_scalar_mul(srowL[:, NB : 2 * NB], srowL[:, 0:NB], 0.5)

        # replicate the var-derived pair {w2, w2h} (critical path: bf16 matmul)
        browL = sm.tile([1, 2 * W], BF16, tag="browL")
        nc.vector.tensor_copy(
            browL[:].rearrange("q (s t) -> q s t", t=T),
            srowL[:].unsqueeze(2).broadcast_to([1, 2 * NB, T]),
        )
        blkL_ps = psR.tile([P, 2 * W], F32, tag="blkLps")
        nc.tensor.matmul(blkL_ps[:], onrb[:], browL[:], start=True, stop=True)
        blkL = bat.tile([P, 2 * W], F32, tag="blkL")
        nc.scalar.copy(blkL[:], blkL_ps[:])
        w2b = blkL[:, 0:W]
        w2hb = blkL[:, W : 2 * W]

        nn = bt("nn")
        nc.vector.tensor_mul(nn[:], dd[:], w2b[:])
        nc.vector.tensor_scalar_max(nn[:], nn[:], SQRT_EPS)
        ee = bt("ee")
        nc.scalar.activation(ee[:], nn[:], AF.Exp)
        em = bt("em")
        nc.scalar.activation(em[:], nn[:], AF.Exp, scale=-1.0)
        rn = bt("rn")
        nc.vector.reciprocal(rn[:], nn[:])
        sh = bt("sh")
        nc.vector.tensor_sub(sh[:], ee[:], em[:])
        ch = bt("ch")
        nc.vector.tensor_add(ch[:], ee[:], em[:])
        sc = bt("sc")
        nc.vector.tensor_mul(sc[:], sh[:], rn[:])       # 2*sinh(n)/n
        scW = bt("scW")
        nc.vector.tensor_mul(scW[:], sc[:], w2hb[:])    # sc*w2/2

        bet = bt("bet")
        nc.vector.tensor_scalar_mul(bet[:], x0_all[:], float(-bm0))
        t1 = bt("t1")
        nc.vector.tensor_mul(t1[:], al_all[:], gnegb[:])
        nc.vector.tensor_add(t1[:], t1[:], bet[:])
        k1 = bt("k1")
        nc.vector.tensor_mul(k1[:], t1[:], invdb[:])
        kf = bt("kf")
        nc.vector.tensor_mul(kf[:], k1[:], c1[:])

        Af = bt("Af")
        nc.vector.tensor_mul(Af[:], scW[:], c1[:])
        Cpre = bt("Cpre")
        nc.vector.tensor_mul(Cpre[:], scW[:], kf[:])
        # B = scW*(kf - c1*alpha) = Cpre - Af*alpha
        Bf = bt("Bf")
        nc.vector.tensor_mul(Bf[:], Af[:], al_all[:])
        nc.vector.tensor_sub(Bf[:], Cpre[:], Bf[:])
        Cf = bt("Cf")
        nc.vector.scalar_tensor_tensor(
            out=Cf[:], in0=ch[:], scalar=0.5, in1=Cpre[:], op0=OP.mult, op1=OP.add
        )
        Ab = bat.tile([P, W], BF16, tag="Ab")
        nc.vector.tensor_copy(Ab[:], Af[:])
        Bb = bat.tile([P, W], BF16, tag="Bb")
        nc.vector.tensor_copy(Bb[:], Bf[:])
        Cb = bat.tile([P, W], BF16, tag="Cb")
        nc.vector.tensor_copy(Cb[:], Cf[:])

        # =========== phase C: per-slab combine + store ===========
        for b in range(NB):
            bs = slice(b * T, (b + 1) * T)
            xb3 = xb_t[b].rearrange("p (t d) -> p t d", d=D)
            # dense bf16 expansions of the A/B columns (ACT, off DVE)
            aexp = esp.tile([P, TD], BF16, tag="aexp")
            a3 = aexp[:].rearrange("p (t d) -> p t d", d=D)
            nc.scalar.copy(a3, Ab[:, bs].unsqueeze(2).broadcast_to([P, T, D]))
            bexp = esp.tile([P, TD], BF16, tag="bexp")
            b3 = bexp[:].rearrange("p (t d) -> p t d", d=D)
            nc.gpsimd.tensor_copy(b3, Bb[:, bs].unsqueeze(2).broadcast_to([P, T, D]))

            r = hsp.tile([P, TD], BF16, tag="r")
            r3 = r[:].rearrange("p (t d) -> p t d", d=D)
            mu_b = mur_t[b].unsqueeze(1).broadcast_to([P, T, D])
            nc.vector.tensor_tensor(r3, b3, mu_b, OP.mult)       # 2x
            out_sb = osp.tile([P, TD], BF16, tag="o")
            o3 = out_sb[:].rearrange("p (t d) -> p t d", d=D)
            nc.vector.tensor_tensor(o3, xb3, a3, OP.mult)        # 2x
            nc.vector.tensor_add(out_sb[:], out_sb[:], r[:])     # 2x
            nc.vector.scalar_tensor_tensor(
                out=o3[:, :, 0], in0=Cb[:, bs], scalar=float(bm0), in1=o3[:, :, 0],
                op0=OP.mult, op1=OP.add,
            )
            nc.sync.dma_start(y_r[b], out_sb[:])

    real_tables = bacc.get_activation_tables
    bacc.get_activation_tables = _only_nat_log_exp_tables
    try:
        nc.compile()
    finally:
        bacc.get_activation_tables = real_tables
    return nc


# ---------------------------------------------------------------------------
# v1 builder (general-bias fallback) -- identical to the original baseline.
# ---------------------------------------------------------------------------

def build_kernel_v1(n_batch: int, has_bias: bool, bm0: float):
    nc = bacc.Bacc("TRN2", target_bir_lowering=False, debug=False)

    x_d = nc.dram_tensor("x", [n_batch, N, D], F32, kind="ExternalInput")
    bm_d = nc.dram_tensor("bm", [1, D], F32, kind="ExternalInput")
    bmt_d = nc.dram_tensor("bmt", [1, D], F32, kind="ExternalInput")
    w_d = nc.dram_tensor("w", [1, 1], F32, kind="ExternalInput")
    onc_d = nc.dram_tensor("ones_col", [P, 1], F32, kind="ExternalInput")
    onr_d = nc.dram_tensor("ones_row", [1, P], F32, kind="ExternalInput")
    onrb_d = nc.dram_tensor("ones_rowb", [1, P], BF16, kind="ExternalInput")
    idn_d = nc.dram_tensor("ident", [P, P], F32, kind="ExternalInput")
    y_d = nc.dram_tensor("y", [n_batch, N, D], F32, kind="ExternalOutput")

    x_r = x_d.ap().rearrange("b (p t) d -> b p (t d)", p=P)
    y_r = y_d.ap().rearrange("b (p t) d -> b p (t d)", p=P)

    with tile.TileContext(nc) as tc, ExitStack() as ctx:
        consts = ctx.enter_context(tc.tile_pool(name="consts", bufs=1))
        xpool = ctx.enter_context(tc.tile_pool(name="xp", bufs=2))
        opool = ctx.enter_context(tc.tile_pool(name="op", bufs=3))
        mpool = ctx.enter_context(tc.tile_pool(name="mp", bufs=2))
        pp = ctx.enter_context(tc.tile_pool(name="pp", bufs=3))
        sm = ctx.enter_context(tc.tile_pool(name="sm", bufs=3))
        btp = ctx.enter_context(tc.tile_pool(name="btp", bufs=2))
        psA = ctx.enter_context(tc.tile_pool(name="psA", bufs=3, space="PSUM"))

        bm = consts.tile([1, D], F32)
        nc.sync.dma_start(bm[:], bm_d.ap())
        bmt = consts.tile([1, D], F32)
        nc.sync.dma_start(bmt[:], bmt_d.ap())
        wgt = consts.tile([1, 1], F32)
        nc.sync.dma_start(wgt[:], w_d.ap())
        onc = consts.tile([P, 1], F32)
        nc.sync.dma_start(onc[:], onc_d.ap())
        onr = consts.tile([1, P], F32)
        nc.sync.dma_start(onr[:], onr_d.ap())
        onrb = consts.tile([1, P], BF16)
        nc.sync.dma_start(onrb[:], onrb_d.ap())
        idn = consts.tile([P, P], F32)
        nc.sync.dma_start(idn[:], idn_d.ap())

        if has_bias:
            bmt_ps = psA.tile([P, D], F32, tag="ps_small")
            nc.tensor.matmul(bmt_ps[:], onr[:], bmt[:], start=True, stop=True)
            bmt_rep = consts.tile([P, D], F32)
            nc.scalar.copy(bmt_rep[:], bmt_ps[:])
            bm_ps = psA.tile([P, D], F32, tag="ps_small")
            nc.tensor.matmul(bm_ps[:], onr[:], bm[:], start=True, stop=True)
            bm_rep = consts.tile([P, D], F32)
            nc.scalar.copy(bm_rep[:], bm_ps[:])

        def stage1(b):
            st = {}
            xb = xpool.tile([P, T * D], F32)
            nc.sync.dma_start(xb[:], x_r[b])
            xb3 = xb[:].rearrange("p (t d) -> p t d", d=D)
            st["xb3"] = xb3
            out_sb = opool.tile([P, T * D], F32)
            st["out_sb"] = out_sb
            h13 = out_sb[:].rearrange("p (t d) -> p t d", d=D)

            s_ps = psA.tile([1, D], F32, tag="ps_small")
            for t in range(T):
                nc.tensor.matmul(
                    s_ps[:], onc[:], xb3[:, t, :], start=(t == 0), stop=(t == T - 1)
                )
            s_sb = sm.tile([1, D], F32)
            nc.scalar.copy(s_sb[:], s_ps[:])

            scr_d = sm.tile([1, D], F32)
            ssum = sm.tile([1, 1], F32)
            nc.vector.tensor_mul(scr_d[:], s_sb[:], s_sb[:])
            nc.vector.tensor_reduce(
                ssum[:], scr_d[:], axis=mybir.AxisListType.X, op=OP.add
            )
            s0sq = sm.tile([1, 1], F32)
            nc.scalar.square(s0sq[:], s_sb[0:1, 0:1])
            nls = sm.tile([1, 1], F32)
            nc.vector.scalar_tensor_tensor(
                out=nls[:], in0=s0sq[:], scalar=2.0, in1=ssum[:],
                op0=OP.mult, op1=OP.subtract,
            )
            nc.vector.tensor_scalar_max(nls[:], nls[:], EPS)
            rls = sm.tile([1, 1], F32)
            nc.vector.reciprocal(rls[:], nls[:])
            rsq = sm.tile([1, 1], F32)
            nc.scalar.sqrt(rsq[:], rls[:])
            mu = sm.tile([1, D], F32)
            nc.vector.tensor_scalar_mul(mu[:], s_sb[:], rsq[:])

            mu_ps = psA.tile([P, D], F32, tag="ps_small")
            nc.tensor.matmul(mu_ps[:], onr[:], mu[:], start=True, stop=True)
            mu_rep = mpool.tile([P, D], F32)
            nc.scalar.copy(mu_rep[:], mu_ps[:])

            stageA = sm.tile([1, 3], F32)
            nc.scalar.mul(stageA[:, 0:1], mu[0:1, 0:1], 2.0)
            scr_d2 = sm.tile([1, D], F32)
            nc.vector.tensor_mul(scr_d2[:], mu[:], bmt[:])
            g_pos = sm.tile([1, 1], F32)
            nc.vector.tensor_reduce(
                g_pos[:], scr_d2[:], axis=mybir.AxisListType.X, op=OP.add
            )
            nc.scalar.mul(stageA[:, 1:2], g_pos[:], -1.0)
            one_mg = sm.tile([1, 1], F32)
            nc.scalar.activation(one_mg[:], g_pos[:], AF.Identity, scale=-1.0, bias=1.0)
            nc.vector.reciprocal(stageA[:, 2:3], one_mg[:])
            repsA_ps = psA.tile([P, 3], F32, tag="ps_small")
            nc.tensor.matmul(repsA_ps[:], onr[:], stageA[:], start=True, stop=True)
            repsA = pp.tile([P, 3], F32)
            nc.scalar.copy(repsA[:], repsA_ps[:])
            mu0x2_rep = repsA[:, 0:1]
            ngam_rep = repsA[:, 1:2]
            invden_rep = repsA[:, 2:3]

            mu_b = mu_rep[:].unsqueeze(1).broadcast_to([P, T, D])
            nc.vector.tensor_tensor(h13, xb3, mu_b, OP.mult)

            pdot = pp.tile([P, T], F32)
            scrA = sm.tile([P, D], F32)
            nc.vector.tensor_reduce(
                pdot[:], h13, axis=mybir.AxisListType.X, op=OP.add
            )

            x0t = pp.tile([P, T], F32)
            nc.scalar.copy(x0t[:], xb3[:, :, 0])
            alpha = pp.tile([P, T], F32)
            nc.vector.scalar_tensor_tensor(
                out=alpha[:], in0=x0t[:], scalar=mu0x2_rep, in1=pdot[:],
                op0=OP.mult, op1=OP.subtract,
            )
            nc.vector.tensor_scalar_max(alpha[:], alpha[:], 1.0 + EPS)

            sq = pp.tile([P, T], F32)
            nc.scalar.square(sq[:], alpha[:])
            am1 = pp.tile([P, T], F32)
            nc.vector.tensor_scalar_add(am1[:], sq[:], -1.0)
            nc.vector.tensor_scalar_max(am1[:], am1[:], EPS)
            nu = pp.tile([P, T], F32)
            nc.scalar.sqrt(nu[:], am1[:])
            dsum = pp.tile([P, T], F32)
            nc.vector.tensor_add(dsum[:], alpha[:], nu[:])
            dd = pp.tile([P, T], F32)
            nc.scalar.activation(dd[:], dsum[:], AF.Ln)
            rnu = pp.tile([P, T], F32)
            nc.vector.reciprocal(rnu[:], nu[:])
            c1 = pp.tile([P, T], F32)
            nc.vector.tensor_mul(c1[:], dd[:], rnu[:])

            scrT = pp.tile([P, T], F32)
            ds1 = pp.tile([P, 1], F32)
            nc.scalar.activation(scrT[:], dd[:], AF.Square, accum_out=ds1[:])
            var_ps = psA.tile([1, 1], F32, tag="ps_small")
            nc.tensor.matmul(var_ps[:], onc[:], ds1[:], start=True, stop=True)
            varm = sm.tile([1, 1], F32)
            nc.scalar.activation(
                varm[:], var_ps[:], AF.Copy, bias=1e-6, scale=1.0 / float(N)
            )
            rv = sm.tile([1, 1], F32)
            nc.vector.reciprocal(rv[:], varm[:])
            w2sq = sm.tile([1, 1], F32)
            nc.vector.tensor_mul(w2sq[:], rv[:], wgt[:])
            stageB = sm.tile([1, 2], F32)
            nc.scalar.sqrt(stageB[:, 0:1], w2sq[:])
            nc.scalar.mul(stageB[:, 1:2], stageB[:, 0:1], 0.5)
            repsB_ps = psA.tile([P, 2], F32, tag="ps_small")
            nc.tensor.matmul(repsB_ps[:], onr[:], stageB[:], start=True, stop=True)
            repsB = pp.tile([P, 2], F32)
            nc.scalar.copy(repsB[:], repsB_ps[:])
            w2_rep = repsB[:, 0:1]
            w2h_rep = repsB[:, 1:2]

            bet = pp.tile([P, T], F32)
            if has_bias:
                hb = btp.tile([P, T * D], F32, tag="hb")
                hb3 = hb[:].rearrange("p (t d) -> p t d", d=D)
                bmt_b = bmt_rep[:].unsqueeze(1).broadcast_to([P, T, D])
                nc.vector.tensor_tensor(hb3, xb3, bmt_b, OP.mult)
                for t in range(T):
                    nc.scalar.activation(
                        scrA[:], hb3[:, t, :], AF.Copy, accum_out=bet[:, t : t + 1]
                    )
            else:
                nc.vector.tensor_scalar_mul(bet[:], x0t[:], float(-bm0))

            t1 = pp.tile([P, T], F32)
            nc.vector.scalar_tensor_tensor(
                out=t1[:], in0=alpha[:], scalar=ngam_rep, in1=bet[:],
                op0=OP.mult, op1=OP.add,
            )
            k1 = pp.tile([P, T], F32)
            nc.vector.tensor_scalar_mul(k1[:], t1[:], invden_rep)
            kf = pp.tile([P, T], F32)
            nc.vector.tensor_mul(kf[:], k1[:], c1[:])

            nn = pp.tile([P, T], F32)
            nc.vector.tensor_scalar_mul(nn[:], dd[:], w2_rep)
            nc.vector.tensor_scalar_max(nn[:], nn[:], SQRT_EPS)
            ee = pp.tile([P, T], F32)
            nc.scalar.activation(ee[:], nn[:], AF.Exp)
            em = pp.tile([P, T], F32)
            nc.scalar.activation(em[:], nn[:], AF.Exp, scale=-1.0)
            rn = pp.tile([P, T], F32)
            nc.vector.reciprocal(rn[:], nn[:])
            sh = pp.tile([P, T], F32)
            nc.vector.tensor_sub(sh[:], ee[:], em[:])
            sc = pp.tile([P, T], F32)
            nc.vector.tensor_mul(sc[:], sh[:], rn[:])
            ch = pp.tile([P, T], F32)
            nc.vector.tensor_add(ch[:], ee[:], em[:])

            Aco = pp.tile([P, T], F32)
            a3 = pp.tile([P, T], F32)
            nc.vector.tensor_scalar_mul(a3[:], c1[:], w2h_rep)
            nc.vector.tensor_mul(Aco[:], sc[:], a3[:])
            st["Aco"] = Aco

            ca = pp.tile([P, T], F32)
            nc.vector.tensor_mul(ca[:], c1[:], alpha[:])
            kc = pp.tile([P, T], F32)
            nc.vector.tensor_sub(kc[:], kf[:], ca[:])
            b3 = pp.tile([P, T], F32)
            nc.vector.tensor_scalar_mul(b3[:], kc[:], w2h_rep)
            Bco = pp.tile([P, T], F32)
            nc.vector.tensor_mul(Bco[:], sc[:], b3[:])
            c3 = pp.tile([P, T], F32)
            nc.vector.tensor_scalar_mul(c3[:], kf[:], w2h_rep)
            c0 = pp.tile([P, T], F32)
            nc.vector.tensor_mul(c0[:], sc[:], c3[:])
            Cco = pp.tile([P, T], F32)
            nc.vector.scalar_tensor_tensor(
                out=Cco[:], in0=ch[:], scalar=0.5, in1=c0[:],
                op0=OP.mult, op1=OP.add,
            )
            st["Bco"] = Bco
            st["Cco"] = Cco
            st["mu_rep"] = mu_rep
            st["b"] = b
            return st

        def stage2(st):
            xb3 = st["xb3"]
            o3 = st["out_sb"][:].rearrange("p (t d) -> p t d", d=D)
            mu_rep, Aco, Bco, Cco, b = st["mu_rep"], st["Aco"], st["Bco"], st["Cco"], st["b"]
            rr = xpool.tile([P, T * D], F32, tag="rr")
            r3 = rr[:].rearrange("p (t d) -> p t d", d=D)
            A_b = Aco[:].unsqueeze(2).broadcast_to([P, T, D])
            B_b = Bco[:].unsqueeze(2).broadcast_to([P, T, D])
            mu_b2 = mu_rep[:].unsqueeze(1).broadcast_to([P, T, D])
            nc.vector.tensor_tensor(r3, B_b, mu_b2, OP.mult)
            nc.vector.tensor_tensor(o3, xb3, A_b, OP.mult)
            nc.vector.tensor_tensor(o3, o3, r3, OP.add)
            if has_bias:
                C_b = Cco[:].unsqueeze(2).broadcast_to([P, T, D])
                bm_b = bm_rep[:].unsqueeze(1).broadcast_to([P, T, D])
                nc.vector.tensor_tensor(r3, C_b, bm_b, OP.mult)
                nc.vector.tensor_tensor(o3, o3, r3, OP.add)
            else:
                nc.vector.scalar_tensor_tensor(
                    out=o3[:, :, 0], in0=Cco[:], scalar=float(bm0), in1=o3[:, :, 0],
                    op0=OP.mult, op1=OP.add,
                )
            nc.sync.dma_start(y_r[b], st["out_sb"][:])

        prev = None
        for b in range(n_batch):
            cur = stage1(b)
            if prev is not None:
                stage2(prev)
            prev = cur
        stage2(prev)

    nc.compile()
    return nc


def _host_bias_manifold(bias: np.ndarray):
    b32 = np.asarray(bias, dtype=np.float32)
    sq = np.float32(np.sum(b32 * b32, dtype=np.float32))
    nrm2 = np.maximum(sq, np.float32(EPS))
    n = np.sqrt(nrm2)
    bm = np.zeros(D, dtype=np.float32)
    bm[0] = np.cosh(n)
    bm[1:] = (np.sinh(n) / n) * b32
    return bm


_CACHE = {}


def _get_nc_v2(bm0):
    if "v2" not in _CACHE:
        _CACHE["v2"] = build_kernel_v2(bm0)
    return _CACHE["v2"]


def _get_nc_v1(n_batch, has_bias, bm0):
    key = ("v1", n_batch, has_bias)
    if key not in _CACHE:
        _CACHE[key] = build_kernel_v1(n_batch, has_bias, bm0)
    return _CACHE[key]


def _make_in_maps_v2(x, weight):
    xb = x.astype(ml_dtypes.bfloat16)
    b_sh = x.shape[0] // N_CORES
    common = {
        "w": np.asarray(weight, dtype=np.float32).reshape(1, 1),
        "ones_col": np.ones((P, 1), dtype=ml_dtypes.bfloat16),
        "ones_colf": np.ones((P, 1), dtype=np.float32),
        "ones_row": np.ones((1, P), dtype=np.float32),
        "ones_rowb": np.ones((1, P), dtype=ml_dtypes.bfloat16),
    }
    return [
        {"x": np.ascontiguousarray(xb[c * b_sh : (c + 1) * b_sh]), **common}
        for c in range(N_CORES)
    ]


def _make_in_maps_v1(x, bias, weight):
    bias = np.asarray(bias, dtype=np.float32)
    bm = _host_bias_manifold(bias)
    bmt = bm.copy()
    bmt[0] = -bmt[0]
    b_sh = x.shape[0] // N_CORES
    common = {
        "bm": bm.reshape(1, D),
        "bmt": bmt.reshape(1, D),
        "w": np.asarray(weight, dtype=np.float32).reshape(1, 1),
        "ones_col": np.ones((P, 1), dtype=np.float32),
        "ones_row": np.ones((1, P), dtype=np.float32),
        "ident": np.eye(P, dtype=np.float32),
    }
    return [
        {"x": np.ascontiguousarray(x[c * b_sh : (c + 1) * b_sh]), **common}
        for c in range(N_CORES)
    ]


def kernel(x, bias, weight):
    from concourse.bass_utils import run_bass_kernel_spmd

    x = np.ascontiguousarray(np.asarray(x, dtype=np.float32))
    assert x.shape == (B_FULL, N, D), x.shape
    bias = np.asarray(bias, dtype=np.float32)
    has_bias = bool(np.any(bias != 0))
    bm = _host_bias_manifold(bias)
    if has_bias:
        in_maps = _make_in_maps_v1(x, bias, weight)
        nc = _get_nc_v1(B_FULL // N_CORES, True, float(bm[0]))
    else:
        in_maps = _make_in_maps_v2(x, weight)
        nc = _get_nc_v2(float(bm[0]))
    res = run_bass_kernel_spmd(nc, in_maps, core_ids=list(range(N_CORES)))
    y = np.concatenate(
        [res.results[c]["y"].astype(np.float32) for c in range(N_CORES)], axis=0
    )
    return y
